# revision 7
# baseline (speedup 1.0000x reference)
# Trainium2 Bass kernel for nn_Block_ViT (4-branch channel-attention ViT block).
#
# Sharding over 8 cores: core c = 2*b + g handles batch b (of 4) and heads
# {2g, 2g+1} (of 4).  Each core computes K/V/Q projections, channel-attention
# scores, instance-norm + softmax, and its 2-head partial context for all 4
# branches.  Per-branch 2-core ReduceScatters sum the context over heads and
# hand each core one 392-token half; Wo + residual + FFN run token-parallel.
# Host-side prep only reshapes/transposes weights and folds LN affine params
# into adjacent matmuls (algebraically exact).
import sys

sys.path.insert(0, "/opt/trn_rl_repo")

import numpy as np
import ml_dtypes

import concourse.bass as bass
import concourse.tile as tile
from concourse import bacc, mybir
from concourse.bass_utils import run_bass_kernel_spmd
from concourse.masks import make_identity

f32 = mybir.dt.float32
f32r = mybir.dt.float32r
bf16 = mybir.dt.bfloat16
AF = mybir.ActivationFunctionType
ALU = mybir.AluOpType

H = 4
B = 4
NTOK = 784
CS = [64, 128, 256, 512]
COFF = [0, 64, 192, 448]
KV = 960
EPS_LN = 1e-6
EPS_IN = 1e-5
SCALE = 1.0 / float(np.sqrt(np.float32(KV)))

TP = 112          # token tile (attention phase), 7 tiles
NT = 7
NH = 392          # tokens per core after ReduceScatter
MP = 98           # token tile (FFN phase), 4 tiles
MT = 4
KT = [(k * 128, min(128, KV - k * 128)) for k in range(8)]   # 960 = 7*128 + 64
RG = [[0, 1], [2, 3], [4, 5], [6, 7]]
JSP = [(0, 512), (512, 448)]  # bank-aligned split of 960
BORD = [3, 2, 1, 0]           # branch order: biggest first

DT = []
for C in CS:
    DT.append([(t * 128, min(128, C - t * 128)) for t in range((C + 127) // 128)])
GDT = []
for i, C in enumerate(CS):
    for (d0, dp) in DT[i]:
        GDT.append((i, d0, dp, COFF[i] + d0))
GBASE = [sum(len(DT[ii]) for ii in range(i)) for i in range(4)]


def _segments(c0, C):
    segs = []
    r = 0
    while r < C:
        g = c0 + r
        kt = g // 128
        k0 = g - kt * 128
        n = min(KT[kt][1] - k0, C - r, 128 - (r % 128))
        segs.append((kt, k0, r, n))
        r += n
    return segs


def _bcast_ap(src_ap, extra_offset, ap):
    return bass.AP(tensor=src_ap.tensor, offset=src_ap.offset + extra_offset, ap=ap)


def build_graph():
    nc = bacc.Bacc(None, target_bir_lowering=False)

    emb = nc.declare_dram_parameter("emb", [NTOK, KV], f32, isOutput=False)
    embT = nc.declare_dram_parameter("embT", [KV, NTOK], f32, isOutput=False)
    emb_half = nc.declare_dram_parameter("emb_half", [NH, KV], f32, isOutput=False)
    wkv = nc.declare_dram_parameter("wkv", [2, 2, KV, KV], f32, isOutput=False)
    kvb = nc.declare_dram_parameter("kvb", [2, 2, KV], f32, isOutput=False)
    wq = [nc.declare_dram_parameter(f"wq{i}", [2, CS[i], CS[i]], bf16, isOutput=False)
          for i in range(4)]
    qb = [nc.declare_dram_parameter(f"qb{i}", [2, CS[i]], f32, isOutput=False)
          for i in range(4)]
    wo = [nc.declare_dram_parameter(f"wo{i}", [CS[i], CS[i]], f32, isOutput=False)
          for i in range(4)]
    w1 = [nc.declare_dram_parameter(f"w1{i}", [CS[i], 4 * CS[i]], f32, isOutput=False)
          for i in range(4)]
    bias1 = [nc.declare_dram_parameter(f"b1{i}", [4 * CS[i]], f32, isOutput=False)
             for i in range(4)]
    w2 = [nc.declare_dram_parameter(f"w2{i}", [4 * CS[i], CS[i]], f32, isOutput=False)
          for i in range(4)]
    bias2 = [nc.declare_dram_parameter(f"b2{i}", [CS[i]], f32, isOutput=False)
             for i in range(4)]
    outs = [nc.declare_dram_parameter(f"out{i}", [NH, CS[i]], f32, isOutput=True)
            for i in range(4)]

    import contextlib
    with tile.TileContext(nc) as tc, contextlib.ExitStack() as outer:
        const = outer.enter_context(tc.tile_pool(name="const", bufs=1))
        dram = outer.enter_context(tc.tile_pool(name="dram", bufs=1, space="DRAM"))
        cpool = outer.enter_context(tc.tile_pool(name="cpool", bufs=1))

        ident = const.tile([128, 128], f32, tag="ident", name="ident")
        make_identity(nc, ident)
        ones_col = const.tile([128, 1], f32, tag="ones", name="ones")
        nc.vector.memset(ones_col, 1.0)
        scale_row = const.tile([1, 128], f32, tag="srow", name="srow")
        nc.vector.memset(scale_row, SCALE)
        eps_ln = const.tile([128, 1], f32, tag="epsln", name="epsln")
        nc.vector.memset(eps_ln, EPS_LN)
        eps_in = const.tile([128, 1], f32, tag="epsin", name="epsin")
        nc.vector.memset(eps_in, EPS_IN)

        cc_in = [dram.tile([2 * CS[i], NH], f32, tag=f"ccin{i}", name=f"ccin{i}")
                 for i in range(4)]
        cc_out = [dram.tile([CS[i], NH], f32, tag=f"ccout{i}", name=f"ccout{i}")
                  for i in range(4)]
        stat_dram = dram.tile([10, NTOK], f32, tag="statd", name="statd")
        ctxr = [None] * len(GDT)

        with contextlib.ExitStack() as phA:
            # ------------- persistent attention-phase tiles ----------------
            pA = phA.enter_context(tc.tile_pool(name="pA", bufs=1))
            eaT = [pA.tile([kp, NTOK], f32r, tag=f"eaT{kt}", name=f"eaT{kt}")
                   for kt, (k0, kp) in enumerate(KT)]
            cxT = {}
            for i, C in enumerate(CS):
                cxT[i] = [pA.tile([dp, NTOK], bf16, tag=f"cxT{i}_{t}",
                                  name=f"cxT{i}_{t}")
                          for t, (d0, dp) in enumerate(DT[i])]
            ctx_acc = [pA.tile([dp, NTOK], f32, tag=f"ctxa{g}", name=f"ctxa{g}")
                       for g, (_, _, dp, _) in enumerate(GDT)]

            tiny = phA.enter_context(tc.tile_pool(name="tiny", bufs=4))
            kvw = phA.enter_context(tc.tile_pool(name="kvw", bufs=9))

            ps_misc = phA.enter_context(
                tc.tile_pool(name="ps_misc", bufs=2, space="PSUM"))
            ps_proj = phA.enter_context(
                tc.tile_pool(name="ps_proj", bufs=2, space="PSUM"))
            ps_sc = phA.enter_context(
                tc.tile_pool(name="ps_sc", bufs=1, space="PSUM"))
            ps_ctx = phA.enter_context(
                tc.tile_pool(name="ps_ctx", bufs=2, space="PSUM"))

            # hoisted: head-0 K weights stream in during the LN phase
            wk0_sb = []
            for kt, (k0, kp) in enumerate(KT):
                t = kvw.tile([kp, KV], f32r, tag="kvw", name=f"wk0_{kt}")
                nc.sync.dma_start(out=t, in_=wkv[0, 0, k0:k0 + kp, :].bitcast(f32r))
                wk0_sb.append(t)

            # ------------- Phase A: LayerNorms in transposed domain --------
            ln_stack = contextlib.ExitStack()
            pLN = ln_stack.enter_context(tc.tile_pool(name="pLN", bufs=1))
            emb_pool = ln_stack.enter_context(tc.tile_pool(name="embp", bufs=3))
            ln_tmp = ln_stack.enter_context(tc.tile_pool(name="lntmp", bufs=3))
            bcastp = ln_stack.enter_context(tc.tile_pool(name="bcastp", bufs=3))
            embT_sb = []
            for kt, (k0, kp) in enumerate(KT):
                t = pLN.tile([kp, NTOK], f32, tag=f"embT{kt}", name=f"embT{kt}")
                nc.sync.dma_start(out=t, in_=embT[k0:k0 + kp, :])
                embT_sb.append(t)
            stat_rows = pLN.tile([10, NTOK], f32, tag="strow", name="strow")

            for it in range(NT):
                n0 = it * TP
                et = emb_pool.tile([TP, KV], f32, tag="emb", name="emb")
                nc.sync.dma_start(out=et, in_=emb[n0:n0 + TP, :])
                stats_m = tiny.tile([TP, 5], f32, tag="stm", name="stm")
                stats_v = tiny.tile([TP, 5], f32, tag="stv", name="stv")
                # ea stats: bn_stats over two equal 480 subgroups
                bste = tiny.tile([TP, 2, 6], f32, tag="bste", name="bste")
                nc.vector.bn_stats(out=bste[:, 0, :], in_=et[:, 0:480])
                nc.vector.bn_stats(out=bste[:, 1, :], in_=et[:, 480:960])
                mve = tiny.tile([TP, 2], f32, tag="mve", name="mve")
                nc.vector.bn_aggr(out=mve, in_=bste)
                nc.vector.tensor_copy(stats_m[:, 0:1], mve[:, 0:1])
                nc.vector.tensor_copy(stats_v[:, 0:1], mve[:, 1:2])
                for i, C in enumerate(CS):
                    c0 = COFF[i]
                    bst = tiny.tile([TP, 6], f32, tag="bst", name="bst")
                    nc.vector.bn_stats(out=bst, in_=et[:, c0:c0 + C])
                    mv = tiny.tile([TP, 2], f32, tag="mv", name="mv")
                    nc.vector.bn_aggr(out=mv, in_=bst)
                    nc.vector.tensor_copy(stats_m[:, i + 1:i + 2], mv[:, 0:1])
                    nc.vector.tensor_copy(stats_v[:, i + 1:i + 2], mv[:, 1:2])
                stds = tiny.tile([TP, 5], f32, tag="stds", name="stds")
                nc.scalar.activation(stds, stats_v, AF.Sqrt, bias=eps_ln[:TP])
                rstds = tiny.tile([TP, 5], f32, tag="rstds", name="rstds")
                nc.vector.reciprocal(rstds, stds)
                pack = tiny.tile([TP, 10], f32, tag="pack", name="pack")
                nc.vector.tensor_copy(pack[:, 0:5], stats_m)
                nc.vector.tensor_copy(pack[:, 5:10], rstds)
                ps_t = ps_misc.tile([10, TP], f32, tag="mips", name="mips")
                nc.tensor.transpose(ps_t, pack, ident[:TP, :TP])
                nc.vector.tensor_copy(stat_rows[:, n0:n0 + TP], ps_t)
                nc.sync.dma_start(out=stat_dram[:, n0:n0 + TP],
                                  in_=stat_rows[:, n0:n0 + TP])
                bc = bcastp.tile([128, 10, TP], f32, tag="bc", name="bc")
                nc.sync.dma_start(
                    out=bc,
                    in_=_bcast_ap(stat_dram, n0, [[0, 128], [NTOK, 10], [1, TP]]))
                for kt, (k0, kp) in enumerate(KT):
                    tmp = ln_tmp.tile([128, TP], f32, tag="apt", name="apt")
                    nc.vector.tensor_sub(tmp[:kp, :], embT_sb[kt][:, n0:n0 + TP],
                                         bc[:kp, 0, :])
                    nc.vector.tensor_mul(eaT[kt][:, n0:n0 + TP], tmp[:kp, :],
                                         bc[:kp, 5, :])
                for i, C in enumerate(CS):
                    for (kt, kk0, br, nr) in _segments(COFF[i], C):
                        bt = br // 128
                        r0 = br - bt * 128
                        tmp = ln_tmp.tile([128, TP], f32, tag="apt", name="apt")
                        nc.vector.tensor_sub(
                            tmp[:nr, :], embT_sb[kt][kk0:kk0 + nr, n0:n0 + TP],
                            bc[kk0:kk0 + nr, 1 + i, :])
                        nc.vector.tensor_mul(
                            cxT[i][bt][r0:r0 + nr, n0:n0 + TP], tmp[:nr, :],
                            bc[:nr, 6 + i, :])

            # ------------- Phase A2: attention, branch-pipelined -----------
            ln_stack.close()
            kpool = phA.enter_context(tc.tile_pool(name="kpool", bufs=1))
            vpool = phA.enter_context(tc.tile_pool(name="vpool", bufs=1))
            qpool = phA.enter_context(tc.tile_pool(name="qpool", bufs=1))
            wqp = phA.enter_context(tc.tile_pool(name="wqp", bufs=1))
            scp = phA.enter_context(tc.tile_pool(name="scp", bufs=1))
            expp = phA.enter_context(tc.tile_pool(name="expp", bufs=4))
            ptp = phA.enter_context(tc.tile_pool(name="ptp", bufs=1))
            p0p = phA.enter_context(tc.tile_pool(name="p0p", bufs=4))
            bvp = phA.enter_context(tc.tile_pool(name="bvp", bufs=1))

            def emit_q_scores(l, i, K_sb):
                C = CS[i]
                ndt = len(DT[i])
                wq_sb = []
                for ct, (c0, cp) in enumerate(DT[i]):
                    t = wqp.tile([cp, C], bf16, tag=f"wq{ct}", name=f"wq{ct}")
                    nc.sync.dma_start(out=t, in_=wq[i][l, c0:c0 + cp, :])
                    wq_sb.append(t)
                qb_bc = bvp.tile([128, C], f32, tag="qbbc", name="qbbc")
                nc.sync.dma_start(out=qb_bc,
                                  in_=_bcast_ap(qb[i][l], 0, [[0, 128], [1, C]]))
                Q_sb = [qpool.tile([TP, C], bf16, tag=f"q{m}", name=f"q{m}")
                        for m in range(NT)]
                for m in range(NT):
                    ps = ps_proj.tile([TP, 512], f32, tag="proj", name="proj")
                    for ct, (c0, cp) in enumerate(DT[i]):
                        nc.tensor.matmul(
                            ps[:, :C], cxT[i][ct][:, m * TP:(m + 1) * TP],
                            wq_sb[ct], start=(ct == 0), stop=(ct == ndt - 1))
                    nc.vector.tensor_add(Q_sb[m], ps[:, :C], qb_bc[:TP, :])
                sc_sb = []
                stack = tiny.tile([128, 2 * ndt], f32, tag="stack", name="stack")
                for dt, (d0, dp) in enumerate(DT[i]):
                    ps_s = ps_sc.tile([dp, 1024], f32, tag="sc", name="sc")
                    for (j0, jn) in JSP:
                        for m in range(NT):
                            nc.tensor.matmul(
                                ps_s[:, j0:j0 + jn], Q_sb[m][:, d0:d0 + dp],
                                K_sb[m][:, j0:j0 + jn],
                                start=(m == 0), stop=(m == NT - 1))
                    bsts = tiny.tile([dp, 2, 6], f32, tag="bsts", name="bsts")
                    nc.vector.bn_stats(out=bsts[:, 0, :], in_=ps_s[:, 0:480])
                    nc.vector.bn_stats(out=bsts[:, 1, :], in_=ps_s[:, 480:960])
                    mvs = tiny.tile([dp, 2], f32, tag="mvs", name="mvs")
                    nc.vector.bn_aggr(out=mvs, in_=bsts)
                    nc.vector.tensor_copy(stack[:dp, dt:dt + 1], mvs[:, 0:1])
                    # E[x^2] = mean^2 + var
                    nc.vector.scalar_tensor_tensor(
                        out=stack[:dp, ndt + dt:ndt + dt + 1], in0=mvs[:, 0:1],
                        scalar=mvs[:, 0:1], in1=mvs[:, 1:2],
                        op0=ALU.mult, op1=ALU.add)
                    sc_t = scp.tile([dp, 1024], f32, tag=f"sct{dt}",
                                    name=f"sct{dt}")
                    nc.scalar.activation(sc_t[:, 0:960], ps_s[:, 0:960],
                                         AF.Identity)
                    sc_sb.append(sc_t)
                return dict(l=l, i=i, sc_sb=sc_sb, stack=stack)

            def emit_tail(st):
                l, i, sc_sb, stack = st["l"], st["i"], st["sc_sb"], st["stack"]
                VT_sb = st["VT_sb"]
                C = CS[i]
                ndt = len(DT[i])
                dpc = DT[i][0][1]
                ps_st = ps_misc.tile([1, 2 * ndt], f32, tag="mips", name="mips")
                nc.tensor.matmul(ps_st, ones_col[:dpc, :], stack[:dpc, :],
                                 start=True, stop=True)
                p0 = p0p.tile([1, 2 * ndt], f32, tag="p0", name="p0")
                nc.vector.tensor_copy(p0, ps_st)
                s1 = p0p.tile([1, 8], f32, tag="p0b", name="p0b")
                # cols: 0=S_m,1=S_E2,2=m_ns,3=E2n,4=msq2,5=var,6=std,7=a
                nc.vector.tensor_reduce(out=s1[:, 0:1], in_=p0[:, 0:ndt],
                                        axis=mybir.AxisListType.X, op=ALU.add)
                nc.vector.tensor_reduce(out=s1[:, 1:2], in_=p0[:, ndt:2 * ndt],
                                        axis=mybir.AxisListType.X, op=ALU.add)
                nc.vector.tensor_scalar(out=s1[:, 2:3], in0=s1[:, 0:1],
                                        scalar1=1.0 / C, scalar2=None,
                                        op0=ALU.mult)
                nc.vector.tensor_scalar(out=s1[:, 3:4], in0=s1[:, 1:2],
                                        scalar1=1.0 / C, scalar2=None,
                                        op0=ALU.mult)
                nc.vector.tensor_tensor(out=s1[:, 4:5], in0=s1[:, 2:3],
                                        in1=s1[:, 2:3], op=ALU.mult)
                nc.vector.tensor_scalar(out=s1[:, 4:5], in0=s1[:, 4:5],
                                        scalar1=SCALE * SCALE, scalar2=None,
                                        op0=ALU.mult)
                nc.vector.tensor_scalar(out=s1[:, 5:6], in0=s1[:, 3:4],
                                        scalar1=SCALE * SCALE,
                                        scalar2=s1[:, 4:5],
                                        op0=ALU.mult, op1=ALU.subtract)
                nc.scalar.activation(s1[:, 6:7], s1[:, 5:6], AF.Sqrt,
                                     bias=eps_in[:1])
                nc.vector.reciprocal(s1[:, 7:8], s1[:, 6:7])
                pair = p0p.tile([1, 2], f32, tag="pair", name="pair")
                nc.vector.tensor_copy(pair[:, 0:1], s1[:, 7:8])
                nc.vector.tensor_scalar(out=pair[:, 1:2], in0=s1[:, 2:3],
                                        scalar1=s1[:, 7:8], scalar2=-1.0,
                                        op0=ALU.mult, op1=ALU.mult)
                ps_ab = ps_misc.tile([128, 2], f32, tag="mips", name="mips")
                nc.tensor.matmul(ps_ab, scale_row, pair, start=True, stop=True)
                ab = tiny.tile([128, 2], f32, tag="ab", name="ab")
                nc.vector.tensor_copy(ab, ps_ab)
                pT = [ptp.tile([jp, C], bf16, tag=f"pt{jt}", name=f"pt{jt}")
                      for jt, (j0, jp) in enumerate(KT)]
                recips = []
                for dt, (d0, dp) in enumerate(DT[i]):
                    ex = expp.tile([dp, 960], f32, tag="exp", name="exp")
                    esum = tiny.tile([dp, 1], f32, tag="esum", name="esum")
                    nc.scalar.activation(ex, sc_sb[dt][:, 0:960], AF.Exp,
                                         bias=ab[:dp, 1:2], scale=ab[:dp, 0:1],
                                         accum_out=esum)
                    recip = tiny.tile([dp, 1], f32, tag="recip", name="recip",
                                      bufs=5)
                    nc.vector.tensor_scalar(out=recip, in0=esum, scalar1=4.0,
                                            scalar2=None, op0=ALU.mult)
                    nc.vector.reciprocal(recip, recip)
                    recips.append(recip)
                    for jt, (j0, jp) in enumerate(KT):
                        ps_t2 = ps_misc.tile([jp, dp], f32, tag="mips",
                                             name="mips")
                        nc.tensor.transpose(ps_t2, ex[:, j0:j0 + jp],
                                            ident[:dp, :dp])
                        nc.vector.tensor_copy(pT[jt][:, d0:d0 + dp], ps_t2)
                for dt, (d0, dp) in enumerate(DT[i]):
                    gdt = GBASE[i] + dt
                    for nh2 in range(2):
                        n0 = nh2 * NH
                        ps_c = ps_ctx.tile([dp, NH], f32, tag="ctx", name="ctx")
                        for jt, (j0, jp) in enumerate(KT):
                            nc.tensor.matmul(
                                ps_c, pT[jt][:, d0:d0 + dp],
                                VT_sb[jt][:, n0:n0 + NH],
                                start=(jt == 0), stop=(jt == 7))
                        if l == 0:
                            nc.vector.tensor_scalar(
                                out=ctx_acc[gdt][:, n0:n0 + NH], in0=ps_c,
                                scalar1=recips[dt], scalar2=None, op0=ALU.mult)
                        else:
                            nc.vector.scalar_tensor_tensor(
                                out=ctx_acc[gdt][:, n0:n0 + NH], in0=ps_c,
                                scalar=recips[dt],
                                in1=ctx_acc[gdt][:, n0:n0 + NH],
                                op0=ALU.mult, op1=ALU.add)
                if l == 1:
                    # fire this branch's ReduceScatter immediately
                    for dt, (d0, dp) in enumerate(DT[i]):
                        gdt = GBASE[i] + dt
                        nc.sync.dma_start(out=cc_in[i][d0:d0 + dp, :],
                                          in_=ctx_acc[gdt][:, 0:NH])
                        nc.sync.dma_start(out=cc_in[i][C + d0:C + d0 + dp, :],
                                          in_=ctx_acc[gdt][:, NH:NTOK])
                    nc.gpsimd.collective_compute(
                        "ReduceScatter", ALU.add, ins=[cc_in[i].opt()],
                        outs=[cc_out[i].opt()], replica_groups=RG)
                    for dt, (d0, dp) in enumerate(DT[i]):
                        gdt = GBASE[i] + dt
                        t = cpool.tile([dp, NH], f32r, tag=f"ctxr{gdt}",
                                       name=f"ctxr{gdt}")
                        nc.sync.dma_start(
                            out=t, in_=cc_out[i][d0:d0 + dp, :].bitcast(f32r))
                        ctxr[gdt] = t

            for l in range(2):
                if l == 0:
                    wk_sb = wk0_sb
                else:
                    wk_sb = []
                    for kt, (k0, kp) in enumerate(KT):
                        t = kvw.tile([kp, KV], f32r, tag="kvw", name=f"wk1_{kt}")
                        nc.sync.dma_start(out=t,
                                          in_=wkv[l, 0, k0:k0 + kp, :].bitcast(f32r))
                        wk_sb.append(t)
                kb_bc = bvp.tile([128, KV], f32, tag="kbbc", name="kbbc")
                nc.sync.dma_start(out=kb_bc,
                                  in_=_bcast_ap(kvb[l, 0], 0, [[0, 128], [1, KV]]))
                K_sb = [kpool.tile([TP, KV], bf16, tag=f"k{m}", name=f"k{m}")
                        for m in range(NT)]
                for m in range(NT):
                    for (j0, jn) in JSP:
                        ps = ps_proj.tile([TP, 512], f32, tag="proj", name="proj")
                        for kt, (k0, kp) in enumerate(KT):
                            nc.tensor.matmul(
                                ps[:, :jn], eaT[kt][:, m * TP:(m + 1) * TP],
                                wk_sb[kt][:, j0:j0 + jn],
                                start=(kt == 0), stop=(kt == 7))
                        nc.vector.tensor_add(K_sb[m][:, j0:j0 + jn], ps[:, :jn],
                                             kb_bc[:TP, j0:j0 + jn])
                wv_sb = []
                for kt, (k0, kp) in enumerate(KT):
                    t = kvw.tile([kp, KV], f32r, tag="kvw", name=f"wv{l}_{kt}")
                    nc.sync.dma_start(out=t,
                                      in_=wkv[l, 1, k0:k0 + kp, :].bitcast(f32r))
                    wv_sb.append(t)
                VT_sb = [vpool.tile([jp, NTOK], bf16, tag=f"v{jt}", name=f"v{jt}")
                         for jt, (j0, jp) in enumerate(KT)]
                for jt, (j0, jp) in enumerate(KT):
                    vb_col = tiny.tile([jp, 1], f32, tag="vbcol", name="vbcol")
                    nc.sync.dma_start(
                        out=vb_col, in_=_bcast_ap(kvb[l, 1], j0, [[1, jp], [0, 1]]))
                    for nh2 in range(2):
                        n0 = nh2 * NH
                        ps = ps_ctx.tile([jp, NH], f32, tag="ctx", name="ctx")
                        for kt, (k0, kp) in enumerate(KT):
                            nc.tensor.matmul(
                                ps, wv_sb[kt][:, j0:j0 + jp],
                                eaT[kt][:, n0:n0 + NH],
                                start=(kt == 0), stop=(kt == 7))
                        nc.vector.tensor_scalar(
                            out=VT_sb[jt][:, n0:n0 + NH], in0=ps, scalar1=vb_col,
                            scalar2=None, op0=ALU.add)

                pending = None
                for i in BORD:
                    st = emit_q_scores(l, i, K_sb)
                    st["VT_sb"] = VT_sb
                    if pending is not None:
                        emit_tail(pending)
                    pending = st
                emit_tail(pending)

        # ---------------- Phase C: Wo + residual + FFN (token-half) --------
        with contextlib.ExitStack() as phC:
            fw1 = phC.enter_context(tc.tile_pool(name="fw1", bufs=1))
            fw2 = phC.enter_context(tc.tile_pool(name="fw2", bufs=1))
            wow = phC.enter_context(tc.tile_pool(name="wow", bufs=1))
            xpool = phC.enter_context(tc.tile_pool(name="xpool", bufs=2))
            hpool = phC.enter_context(tc.tile_pool(name="hpool", bufs=2))
            htp = phC.enter_context(tc.tile_pool(name="htp", bufs=2))
            h2tp = phC.enter_context(tc.tile_pool(name="h2tp", bufs=1))
            ytp = phC.enter_context(tc.tile_pool(name="ytp", bufs=3))
            opool = phC.enter_context(tc.tile_pool(name="opool", bufs=3))
            epool = phC.enter_context(tc.tile_pool(name="epool", bufs=3))
            tinyc = phC.enter_context(tc.tile_pool(name="tinyc", bufs=2))

            pc_o = phC.enter_context(tc.tile_pool(name="pc_o", bufs=2, space="PSUM"))
            pc_h2 = phC.enter_context(tc.tile_pool(name="pc_h2", bufs=2, space="PSUM"))
            pc_y = phC.enter_context(tc.tile_pool(name="pc_y", bufs=2, space="PSUM"))
            pc_tr = phC.enter_context(tc.tile_pool(name="pc_tr", bufs=2, space="PSUM"))

            for i in BORD:
                C = CS[i]
                ndt = len(DT[i])
                gbase = GBASE[i]
                wo_sb = []
                for dt, (d0, dp) in enumerate(DT[i]):
                    t = wow.tile([dp, C], f32r, tag=f"wo{i}_{dt}",
                                 name=f"wo{i}_{dt}")
                    nc.sync.dma_start(out=t, in_=wo[i][d0:d0 + dp, :].bitcast(f32r))
                    wo_sb.append(t)
                w1_sb = []
                for ct, (c0, cp) in enumerate(DT[i]):
                    t = fw1.tile([cp, 4 * C], f32r, tag=f"w1_{i}_{ct}",
                                 name=f"w1_{i}_{ct}")
                    nc.sync.dma_start(out=t, in_=w1[i][c0:c0 + cp, :].bitcast(f32r))
                    w1_sb.append(t)
                JT = [(t * 128, min(128, 4 * C - t * 128))
                      for t in range(4 * C // 128)]
                w2_sb = []
                for jt, (j0, jp) in enumerate(JT):
                    t = fw2.tile([jp, C], f32r, tag=f"w2_{i}_{jt}",
                                 name=f"w2_{i}_{jt}")
                    nc.sync.dma_start(out=t, in_=w2[i][j0:j0 + jp, :].bitcast(f32r))
                    w2_sb.append(t)
                b1_sb = []
                for jt, (j0, jp) in enumerate(JT):
                    t = tinyc.tile([jp, 1], f32, tag=f"b1c{jt}", name=f"b1c{jt}")
                    nc.sync.dma_start(out=t,
                                      in_=_bcast_ap(bias1[i][j0:j0 + jp], 0,
                                                    [[1, jp], [0, 1]]))
                    b1_sb.append(t)
                b2_sb = []
                for ct, (c0, cp) in enumerate(DT[i]):
                    t = tinyc.tile([cp, 1], f32, tag=f"b2c{ct}", name=f"b2c{ct}")
                    nc.sync.dma_start(out=t,
                                      in_=_bcast_ap(bias2[i][c0:c0 + cp], 0,
                                                    [[1, cp], [0, 1]]))
                    b2_sb.append(t)

                x_sb = []
                hT_sb = [htp.tile([cp, NH], f32r, tag=f"ht{ct}", name=f"ht{ct}")
                         for ct, (c0, cp) in enumerate(DT[i])]
                for m in range(MT):
                    m0 = m * MP
                    ps_o = pc_o.tile([MP, C], f32, tag="o", name="o")
                    for dt, (d0, dp) in enumerate(DT[i]):
                        nc.tensor.matmul(ps_o, ctxr[gbase + dt][:, m0:m0 + MP],
                                         wo_sb[dt], start=(dt == 0),
                                         stop=(dt == ndt - 1))
                    e_t = epool.tile([MP, C], f32, tag="e", name="e")
                    nc.sync.dma_start(out=e_t,
                                      in_=emb_half[m0:m0 + MP, COFF[i]:COFF[i] + C])
                    x_t = xpool.tile([MP, C], f32, tag=f"x{m}", name=f"x{m}")
                    nc.vector.tensor_add(x_t, e_t, ps_o)
                    x_sb.append(x_t)
                    bst = tinyc.tile([MP, 6], f32, tag="bstc", name="bstc")
                    nc.vector.bn_stats(out=bst, in_=x_t)
                    mv = tinyc.tile([MP, 2], f32, tag="mvc", name="mvc")
                    nc.vector.bn_aggr(out=mv, in_=bst)
                    stdv = tinyc.tile([MP, 1], f32, tag="stdc", name="stdc")
                    nc.scalar.activation(stdv, mv[:, 1:2], AF.Sqrt,
                                         bias=eps_ln[:MP])
                    rstd = tinyc.tile([MP, 1], f32, tag="rstdc", name="rstdc")
                    nc.vector.reciprocal(rstd, stdv)
                    h_t = hpool.tile([MP, C], f32, tag="h", name="h")
                    nc.vector.tensor_scalar(out=h_t, in0=x_t, scalar1=mv[:, 0:1],
                                            scalar2=rstd, op0=ALU.subtract,
                                            op1=ALU.mult)
                    for ct, (c0, cp) in enumerate(DT[i]):
                        ps_t3 = pc_tr.tile([cp, MP], f32, tag="tr", name="tr")
                        nc.tensor.transpose(ps_t3, h_t[:, c0:c0 + cp],
                                            ident[:MP, :MP])
                        nc.vector.tensor_copy(hT_sb[ct][:, m0:m0 + MP], ps_t3)
                h2T_sb = []
                for jt, (j0, jp) in enumerate(JT):
                    ps_h = pc_h2.tile([jp, NH], f32, tag="h2", name="h2")
                    for ct, (c0, cp) in enumerate(DT[i]):
                        nc.tensor.matmul(ps_h, w1_sb[ct][:, j0:j0 + jp], hT_sb[ct],
                                         start=(ct == 0), stop=(ct == ndt - 1))
                    h2t = h2tp.tile([jp, NH], f32r, tag=f"h2t{jt}",
                                    name=f"h2t{jt}")
                    nc.scalar.activation(h2t, ps_h, AF.Gelu, bias=b1_sb[jt])
                    h2T_sb.append(h2t)
                for ct, (c0, cp) in enumerate(DT[i]):
                    ps_y = pc_y.tile([cp, NH], f32, tag="y", name="y")
                    for jt, (j0, jp) in enumerate(JT):
                        nc.tensor.matmul(ps_y, w2_sb[jt][:, c0:c0 + cp],
                                         h2T_sb[jt],
                                         start=(jt == 0), stop=(jt == len(JT) - 1))
                    yt = ytp.tile([cp, NH], f32, tag="yt", name="yt")
                    nc.scalar.activation(yt, ps_y, AF.Identity, bias=b2_sb[ct])
                    for m in range(MT):
                        m0 = m * MP
                        ps_t4 = pc_tr.tile([MP, cp], f32, tag="tr", name="tr")
                        nc.tensor.transpose(ps_t4, yt[:, m0:m0 + MP],
                                            ident[:cp, :cp])
                        o_t = opool.tile([MP, 128], f32, tag="ot", name="ot")
                        nc.vector.tensor_add(o_t[:, :cp], x_sb[m][:, c0:c0 + cp],
                                             ps_t4)
                        nc.sync.dma_start(out=outs[i][m0:m0 + MP, c0:c0 + cp],
                                          in_=o_t[:, :cp])

    nc.compile()
    return nc


_CACHE = {}


def _get_graph():
    if "nc" not in _CACHE:
        _CACHE["nc"] = build_graph()
    return _CACHE["nc"]


def _prep_core_inputs(inputs, b, g):
    f = np.float32
    emb_cat = np.concatenate(
        [np.asarray(inputs[f"emb{i+1}"][b], dtype=f) for i in range(4)], axis=-1)
    emb_cat = np.ascontiguousarray(emb_cat)
    m = {
        "emb": emb_cat,
        "embT": np.ascontiguousarray(emb_cat.T),
        "emb_half": np.ascontiguousarray(emb_cat[g * NH:(g + 1) * NH]),
    }
    anA_g = np.asarray(inputs["anA_g"], f)
    anA_b = np.asarray(inputs["anA_b"], f)
    wkv_m = np.empty((2, 2, KV, KV), f)
    kvb_m = np.empty((2, 2, KV), f)
    for li in range(2):
        h = 2 * g + li
        Wk = np.asarray(inputs["Wk"][h], f)
        Wv = np.asarray(inputs["Wv"][h], f)
        wkv_m[li, 0] = anA_g[:, None] * Wk.T
        wkv_m[li, 1] = anA_g[:, None] * Wv.T
        kvb_m[li, 0] = anA_b @ Wk.T
        kvb_m[li, 1] = anA_b @ Wv.T
    m["wkv"] = wkv_m
    m["kvb"] = kvb_m
    for i, C in enumerate(CS):
        an_g = np.asarray(inputs[f"an{i+1}_g"], f)
        an_b = np.asarray(inputs[f"an{i+1}_b"], f)
        fn_g = np.asarray(inputs[f"fn{i+1}_g"], f)
        fn_b = np.asarray(inputs[f"fn{i+1}_b"], f)
        Wq = np.asarray(inputs[f"Wq{i+1}"], f)
        wq_i = np.empty((2, C, C), np.float32)
        qb_i = np.empty((2, C), f)
        for li in range(2):
            h = 2 * g + li
            wq_i[li] = an_g[:, None] * Wq[h].T
            qb_i[li] = an_b @ Wq[h].T
        m[f"wq{i}"] = wq_i.astype(ml_dtypes.bfloat16)
        m[f"qb{i}"] = qb_i
        m[f"wo{i}"] = np.ascontiguousarray(np.asarray(inputs[f"Wo{i+1}"], f).T)
        w1_ = np.asarray(inputs[f"fc{i+1}1_w"], f)
        m[f"w1{i}"] = np.ascontiguousarray(fn_g[:, None] * w1_.T)
        m[f"b1{i}"] = np.asarray(inputs[f"fc{i+1}1_b"], f) + w1_ @ fn_b
        m[f"w2{i}"] = np.ascontiguousarray(np.asarray(inputs[f"fc{i+1}2_w"], f).T)
        m[f"b2{i}"] = np.asarray(inputs[f"fc{i+1}2_b"], f)
    return m


def _run(inputs, trace=False):
    nc = _get_graph()
    in_maps = [_prep_core_inputs(inputs, c // 2, c % 2) for c in range(8)]
    res = run_bass_kernel_spmd(nc, in_maps, list(range(8)), trace=trace)
    full = []
    for i, C in enumerate(CS):
        o = np.empty((B, NTOK, C), np.float32)
        for c in range(8):
            b, g = c // 2, c % 2
            o[b, g * NH:(g + 1) * NH, :] = res.results[c][f"out{i}"]
        full.append(o)
    return tuple(full), res


def kernel(**inputs):
    out, _ = _run(inputs, trace=False)
    return out


def kernel_timed(**inputs):
    out, res = _run(inputs, trace=True)
    return out, res.exec_time_ns


# revision 9
# speedup vs baseline: 1.0747x; 1.0747x over previous
# Trainium2 Bass kernel for nn_Block_ViT (4-branch channel-attention ViT block).
#
# Sharding over 8 cores: core c = 2*b + g handles batch b (of 4) and heads
# {2g, 2g+1} (of 4).  Each core computes K/V/Q projections, channel-attention
# scores, instance-norm + softmax, and its 2-head partial context for all 4
# branches.  Per-branch 2-core ReduceScatters sum the context over heads and
# hand each core one 392-token half; Wo + residual + FFN run token-parallel.
# Host-side prep only reshapes/transposes weights and folds LN affine params
# into adjacent matmuls (algebraically exact).
import sys

sys.path.insert(0, "/opt/trn_rl_repo")

import numpy as np
import ml_dtypes

import concourse.bass as bass
import concourse.tile as tile
from concourse import bacc, mybir
from concourse.bass_utils import run_bass_kernel_spmd
from concourse.masks import make_identity

f32 = mybir.dt.float32
f32r = mybir.dt.float32r
bf16 = mybir.dt.bfloat16
AF = mybir.ActivationFunctionType
ALU = mybir.AluOpType

H = 4
B = 4
NTOK = 784
CS = [64, 128, 256, 512]
COFF = [0, 64, 192, 448]
KV = 960
EPS_LN = 1e-6
EPS_IN = 1e-5
SCALE = 1.0 / float(np.sqrt(np.float32(KV)))

TP = 112          # token tile (attention phase), 7 tiles
NT = 7
NH = 392          # tokens per core after ReduceScatter
MP = 98           # token tile (FFN phase), 4 tiles
MT = 4
KT = [(k * 128, min(128, KV - k * 128)) for k in range(8)]   # 960 = 7*128 + 64
RG = [[0, 1], [2, 3], [4, 5], [6, 7]]
JSP = [(0, 512), (512, 448)]  # bank-aligned split of 960
BORD = [3, 2, 1, 0]           # branch order: biggest first

DT = []
for C in CS:
    DT.append([(t * 128, min(128, C - t * 128)) for t in range((C + 127) // 128)])
GDT = []
for i, C in enumerate(CS):
    for (d0, dp) in DT[i]:
        GDT.append((i, d0, dp, COFF[i] + d0))
GBASE = [sum(len(DT[ii]) for ii in range(i)) for i in range(4)]


def _segments(c0, C):
    segs = []
    r = 0
    while r < C:
        g = c0 + r
        kt = g // 128
        k0 = g - kt * 128
        n = min(KT[kt][1] - k0, C - r, 128 - (r % 128))
        segs.append((kt, k0, r, n))
        r += n
    return segs


def _bcast_ap(src_ap, extra_offset, ap):
    return bass.AP(tensor=src_ap.tensor, offset=src_ap.offset + extra_offset, ap=ap)


def build_graph():
    nc = bacc.Bacc(None, target_bir_lowering=False)

    emb = nc.declare_dram_parameter("emb", [NTOK, KV], f32, isOutput=False)
    embT = nc.declare_dram_parameter("embT", [KV, NTOK], f32, isOutput=False)
    emb_half = nc.declare_dram_parameter("emb_half", [NH, KV], f32, isOutput=False)
    wkv = nc.declare_dram_parameter("wkv", [2, 2, KV, KV], f32, isOutput=False)
    kvb = nc.declare_dram_parameter("kvb", [2, 2, KV], f32, isOutput=False)
    wq = [nc.declare_dram_parameter(f"wq{i}", [2, CS[i], CS[i]], bf16, isOutput=False)
          for i in range(4)]
    qb = [nc.declare_dram_parameter(f"qb{i}", [2, CS[i]], f32, isOutput=False)
          for i in range(4)]
    wo = [nc.declare_dram_parameter(f"wo{i}", [CS[i], CS[i]], f32, isOutput=False)
          for i in range(4)]
    w1 = [nc.declare_dram_parameter(f"w1{i}", [CS[i], 4 * CS[i]], bf16, isOutput=False)
          for i in range(4)]
    bias1 = [nc.declare_dram_parameter(f"b1{i}", [4 * CS[i]], f32, isOutput=False)
             for i in range(4)]
    w2 = [nc.declare_dram_parameter(f"w2{i}", [4 * CS[i], CS[i]], bf16, isOutput=False)
          for i in range(4)]
    bias2 = [nc.declare_dram_parameter(f"b2{i}", [CS[i]], f32, isOutput=False)
             for i in range(4)]
    outs = [nc.declare_dram_parameter(f"out{i}", [NH, CS[i]], f32, isOutput=True)
            for i in range(4)]

    import contextlib
    with tile.TileContext(nc) as tc, contextlib.ExitStack() as outer:
        const = outer.enter_context(tc.tile_pool(name="const", bufs=1))
        dram = outer.enter_context(tc.tile_pool(name="dram", bufs=1, space="DRAM"))
        cpool = outer.enter_context(tc.tile_pool(name="cpool", bufs=1))

        ident = const.tile([128, 128], f32, tag="ident", name="ident")
        make_identity(nc, ident)
        ones_col = const.tile([128, 1], f32, tag="ones", name="ones")
        nc.vector.memset(ones_col, 1.0)
        scale_row = const.tile([1, 128], f32, tag="srow", name="srow")
        nc.vector.memset(scale_row, SCALE)
        eps_ln = const.tile([128, 1], f32, tag="epsln", name="epsln")
        nc.vector.memset(eps_ln, EPS_LN)
        eps_in = const.tile([128, 1], f32, tag="epsin", name="epsin")
        nc.vector.memset(eps_in, EPS_IN)

        cc_in = [dram.tile([2 * CS[i], NH], f32, tag=f"ccin{i}", name=f"ccin{i}")
                 for i in range(4)]
        cc_out = [dram.tile([CS[i], NH], f32, tag=f"ccout{i}", name=f"ccout{i}")
                  for i in range(4)]
        stat_dram = dram.tile([10, NTOK], f32, tag="statd", name="statd")
        ctxr = [None] * len(GDT)

        with contextlib.ExitStack() as phA:
            # ------------- persistent attention-phase tiles ----------------
            pA = phA.enter_context(tc.tile_pool(name="pA", bufs=1))
            eaT = [pA.tile([kp, NTOK], f32r, tag=f"eaT{kt}", name=f"eaT{kt}")
                   for kt, (k0, kp) in enumerate(KT)]
            cxT = {}
            for i, C in enumerate(CS):
                cxT[i] = [pA.tile([dp, NTOK], bf16, tag=f"cxT{i}_{t}",
                                  name=f"cxT{i}_{t}")
                          for t, (d0, dp) in enumerate(DT[i])]
            ctx_acc = [pA.tile([dp, NTOK], f32, tag=f"ctxa{g}", name=f"ctxa{g}")
                       for g, (_, _, dp, _) in enumerate(GDT)]

            tiny = phA.enter_context(tc.tile_pool(name="tiny", bufs=4))
            kvw = phA.enter_context(tc.tile_pool(name="kvw", bufs=9))

            ps_misc = phA.enter_context(
                tc.tile_pool(name="ps_misc", bufs=2, space="PSUM"))
            ps_proj = phA.enter_context(
                tc.tile_pool(name="ps_proj", bufs=2, space="PSUM"))
            ps_sc = phA.enter_context(
                tc.tile_pool(name="ps_sc", bufs=1, space="PSUM"))
            ps_ctx = phA.enter_context(
                tc.tile_pool(name="ps_ctx", bufs=2, space="PSUM"))

            # hoisted: head-0 K weights stream in during the LN phase
            wk0_sb = []
            for kt, (k0, kp) in enumerate(KT):
                t = kvw.tile([kp, KV], f32r, tag="kvw", name=f"wk0_{kt}")
                nc.sync.dma_start(out=t, in_=wkv[0, 0, k0:k0 + kp, :].bitcast(f32r))
                wk0_sb.append(t)

            # ------------- Phase A: LayerNorms in transposed domain --------
            ln_stack = contextlib.ExitStack()
            pLN = ln_stack.enter_context(tc.tile_pool(name="pLN", bufs=1))
            emb_pool = ln_stack.enter_context(tc.tile_pool(name="embp", bufs=3))
            ln_tmp = ln_stack.enter_context(tc.tile_pool(name="lntmp", bufs=3))
            bcastp = ln_stack.enter_context(tc.tile_pool(name="bcastp", bufs=3))
            embT_sb = []
            for kt, (k0, kp) in enumerate(KT):
                t = pLN.tile([kp, NTOK], f32, tag=f"embT{kt}", name=f"embT{kt}")
                nc.sync.dma_start(out=t, in_=embT[k0:k0 + kp, :])
                embT_sb.append(t)
            stat_rows = pLN.tile([10, NTOK], f32, tag="strow", name="strow")

            # stats per n-tile; pack interleaved [m,rstd] pairs per group
            for it in range(NT):
                n0 = it * TP
                et = emb_pool.tile([TP, KV], f32, tag="emb", name="emb")
                nc.sync.dma_start(out=et, in_=emb[n0:n0 + TP, :])
                pack = tiny.tile([TP, 5, 2], f32, tag="pack", name="pack")
                bste = tiny.tile([TP, 2, 6], f32, tag="bste", name="bste")
                nc.vector.bn_stats(out=bste[:, 0, :], in_=et[:, 0:480])
                nc.vector.bn_stats(out=bste[:, 1, :], in_=et[:, 480:960])
                nc.vector.bn_aggr(out=pack[:, 0, :], in_=bste)
                for i, C in enumerate(CS):
                    c0 = COFF[i]
                    bst = tiny.tile([TP, 6], f32, tag="bst", name="bst")
                    nc.vector.bn_stats(out=bst, in_=et[:, c0:c0 + C])
                    nc.vector.bn_aggr(out=pack[:, 1 + i, :], in_=bst)
                # rstd in place on the var column of each pair
                nc.scalar.activation(pack[:, :, 1], pack[:, :, 1], AF.Sqrt,
                                     bias=eps_ln[:TP])
                nc.vector.reciprocal(pack[:, :, 1], pack[:, :, 1])
                ps_t = ps_misc.tile([10, TP], f32, tag="mips", name="mips")
                nc.tensor.transpose(ps_t, pack.rearrange("p a b -> p (a b)"),
                                    ident[:TP, :TP])
                nc.vector.tensor_copy(stat_rows[:, n0:n0 + TP], ps_t)
                nc.sync.dma_start(out=stat_dram[:, n0:n0 + TP],
                                  in_=stat_rows[:, n0:n0 + TP])

            # batched applies per n-chunk (448 + 336 tokens)
            for (h0, hn) in ((0, 448), (448, 336)):
                bc = bcastp.tile([128, 10, hn], f32, tag="bc", name="bc",
                                 bufs=2)
                nc.sync.dma_start(
                    out=bc,
                    in_=_bcast_ap(stat_dram, h0, [[0, 128], [NTOK, 10], [1, hn]]))
                for kt, (k0, kp) in enumerate(KT):
                    tmp = ln_tmp.tile([128, hn], f32, tag="apt", name="apt")
                    nc.vector.tensor_sub(tmp[:kp, :], embT_sb[kt][:, h0:h0 + hn],
                                         bc[:kp, 0, :])
                    nc.vector.tensor_mul(eaT[kt][:, h0:h0 + hn], tmp[:kp, :],
                                         bc[:kp, 1, :])
                for i, C in enumerate(CS):
                    for (kt, kk0, br, nr) in _segments(COFF[i], C):
                        bt = br // 128
                        r0 = br - bt * 128
                        tmp = ln_tmp.tile([128, hn], f32, tag="apt", name="apt")
                        nc.vector.tensor_sub(
                            tmp[:nr, :], embT_sb[kt][kk0:kk0 + nr, h0:h0 + hn],
                            bc[kk0:kk0 + nr, 2 + 2 * i, :])
                        nc.vector.tensor_mul(
                            cxT[i][bt][r0:r0 + nr, h0:h0 + hn], tmp[:nr, :],
                            bc[:nr, 3 + 2 * i, :])

            # ------------- Phase A2: attention, branch-pipelined -----------
            ln_stack.close()
            kpool = phA.enter_context(tc.tile_pool(name="kpool", bufs=1))
            vpool = phA.enter_context(tc.tile_pool(name="vpool", bufs=1))
            qpool = phA.enter_context(tc.tile_pool(name="qpool", bufs=1))
            wqp = phA.enter_context(tc.tile_pool(name="wqp", bufs=1))
            scp = phA.enter_context(tc.tile_pool(name="scp", bufs=1))
            expp = phA.enter_context(tc.tile_pool(name="expp", bufs=4))
            ptp = phA.enter_context(tc.tile_pool(name="ptp", bufs=1))
            p0p = phA.enter_context(tc.tile_pool(name="p0p", bufs=4))
            bvp = phA.enter_context(tc.tile_pool(name="bvp", bufs=1))

            def emit_q_scores(l, i, K_sb):
                C = CS[i]
                ndt = len(DT[i])
                wq_sb = []
                for ct, (c0, cp) in enumerate(DT[i]):
                    t = wqp.tile([cp, C], bf16, tag=f"wq{ct}", name=f"wq{ct}")
                    nc.sync.dma_start(out=t, in_=wq[i][l, c0:c0 + cp, :])
                    wq_sb.append(t)
                qb_bc = bvp.tile([128, C], f32, tag="qbbc", name="qbbc")
                nc.sync.dma_start(out=qb_bc,
                                  in_=_bcast_ap(qb[i][l], 0, [[0, 128], [1, C]]))
                Q_sb = [qpool.tile([TP, C], bf16, tag=f"q{m}", name=f"q{m}")
                        for m in range(NT)]
                for m in range(NT):
                    ps = ps_proj.tile([TP, 512], f32, tag="proj", name="proj")
                    for ct, (c0, cp) in enumerate(DT[i]):
                        nc.tensor.matmul(
                            ps[:, :C], cxT[i][ct][:, m * TP:(m + 1) * TP],
                            wq_sb[ct], start=(ct == 0), stop=(ct == ndt - 1))
                    nc.vector.tensor_add(Q_sb[m], ps[:, :C], qb_bc[:TP, :])
                sc_sb = []
                stack = tiny.tile([128, 2 * ndt], f32, tag="stack", name="stack")
                for dt, (d0, dp) in enumerate(DT[i]):
                    ps_s = ps_sc.tile([dp, 1024], f32, tag="sc", name="sc")
                    for (j0, jn) in JSP:
                        for m in range(NT):
                            nc.tensor.matmul(
                                ps_s[:, j0:j0 + jn], Q_sb[m][:, d0:d0 + dp],
                                K_sb[m][:, j0:j0 + jn],
                                start=(m == 0), stop=(m == NT - 1))
                    bsts = tiny.tile([dp, 2, 6], f32, tag="bsts", name="bsts")
                    nc.vector.bn_stats(out=bsts[:, 0, :], in_=ps_s[:, 0:480])
                    nc.vector.bn_stats(out=bsts[:, 1, :], in_=ps_s[:, 480:960])
                    mvs = tiny.tile([dp, 2], f32, tag="mvs", name="mvs")
                    nc.vector.bn_aggr(out=mvs, in_=bsts)
                    nc.vector.tensor_copy(stack[:dp, dt:dt + 1], mvs[:, 0:1])
                    # E[x^2] = mean^2 + var
                    nc.vector.scalar_tensor_tensor(
                        out=stack[:dp, ndt + dt:ndt + dt + 1], in0=mvs[:, 0:1],
                        scalar=mvs[:, 0:1], in1=mvs[:, 1:2],
                        op0=ALU.mult, op1=ALU.add)
                    sc_t = scp.tile([dp, 1024], f32, tag=f"sct{dt}",
                                    name=f"sct{dt}")
                    nc.scalar.activation(sc_t[:, 0:960], ps_s[:, 0:960],
                                         AF.Identity)
                    sc_sb.append(sc_t)
                return dict(l=l, i=i, sc_sb=sc_sb, stack=stack)

            def emit_tail(st):
                l, i, sc_sb, stack = st["l"], st["i"], st["sc_sb"], st["stack"]
                VT_sb = st["VT_sb"]
                C = CS[i]
                ndt = len(DT[i])
                dpc = DT[i][0][1]
                ps_st = ps_misc.tile([1, 2 * ndt], f32, tag="mips", name="mips")
                nc.tensor.matmul(ps_st, ones_col[:dpc, :], stack[:dpc, :],
                                 start=True, stop=True)
                p0 = p0p.tile([1, 2, ndt], f32, tag="p0", name="p0")
                nc.vector.tensor_copy(p0.rearrange("p a b -> p (a b)"), ps_st)
                s1 = p0p.tile([1, 8], f32, tag="p0b", name="p0b")
                # cols: 0=m_ns,1=E2n,2=msq,3=var,4=std,5=a
                nc.vector.tensor_reduce(out=s1[:, 0:2], in_=p0,
                                        axis=mybir.AxisListType.X, op=ALU.add)
                nc.vector.tensor_scalar(out=s1[:, 0:2], in0=s1[:, 0:2],
                                        scalar1=1.0 / C, scalar2=None,
                                        op0=ALU.mult)
                nc.vector.tensor_tensor(out=s1[:, 2:3], in0=s1[:, 0:1],
                                        in1=s1[:, 0:1], op=ALU.mult)
                nc.vector.tensor_scalar(out=s1[:, 3:4], in0=s1[:, 1:2],
                                        scalar1=s1[:, 2:3], scalar2=SCALE * SCALE,
                                        op0=ALU.subtract, op1=ALU.mult)
                nc.scalar.activation(s1[:, 4:5], s1[:, 3:4], AF.Sqrt,
                                     bias=eps_in[:1])
                pair = p0p.tile([1, 2], f32, tag="pair", name="pair")
                nc.vector.reciprocal(pair[:, 0:1], s1[:, 4:5])
                nc.vector.tensor_scalar(out=pair[:, 1:2], in0=s1[:, 0:1],
                                        scalar1=pair[:, 0:1], scalar2=-1.0,
                                        op0=ALU.mult, op1=ALU.mult)
                ps_ab = ps_misc.tile([128, 2], f32, tag="mips", name="mips")
                nc.tensor.matmul(ps_ab, scale_row, pair, start=True, stop=True)
                ab = tiny.tile([128, 2], f32, tag="ab", name="ab")
                nc.vector.tensor_copy(ab, ps_ab)
                pT = [ptp.tile([jp, C], bf16, tag=f"pt{jt}", name=f"pt{jt}")
                      for jt, (j0, jp) in enumerate(KT)]
                recips = []
                for dt, (d0, dp) in enumerate(DT[i]):
                    ex = expp.tile([dp, 960], f32, tag="exp", name="exp")
                    esum = tiny.tile([dp, 1], f32, tag="esum", name="esum")
                    nc.scalar.activation(ex, sc_sb[dt][:, 0:960], AF.Exp,
                                         bias=ab[:dp, 1:2], scale=ab[:dp, 0:1],
                                         accum_out=esum)
                    recip = tiny.tile([dp, 1], f32, tag="recip", name="recip",
                                      bufs=5)
                    nc.vector.tensor_scalar(out=recip, in0=esum, scalar1=4.0,
                                            scalar2=None, op0=ALU.mult)
                    nc.vector.reciprocal(recip, recip)
                    recips.append(recip)
                    for jt, (j0, jp) in enumerate(KT):
                        ps_t2 = ps_misc.tile([jp, dp], f32, tag="mips",
                                             name="mips")
                        nc.tensor.transpose(ps_t2, ex[:, j0:j0 + jp],
                                            ident[:dp, :dp])
                        nc.vector.tensor_copy(pT[jt][:, d0:d0 + dp], ps_t2)
                for dt, (d0, dp) in enumerate(DT[i]):
                    gdt = GBASE[i] + dt
                    for nh2 in range(2):
                        n0 = nh2 * NH
                        ps_c = ps_ctx.tile([dp, NH], f32, tag="ctx", name="ctx")
                        for jt, (j0, jp) in enumerate(KT):
                            nc.tensor.matmul(
                                ps_c, pT[jt][:, d0:d0 + dp],
                                VT_sb[jt][:, n0:n0 + NH],
                                start=(jt == 0), stop=(jt == 7))
                        if l == 0:
                            nc.vector.tensor_scalar(
                                out=ctx_acc[gdt][:, n0:n0 + NH], in0=ps_c,
                                scalar1=recips[dt], scalar2=None, op0=ALU.mult)
                        else:
                            nc.vector.scalar_tensor_tensor(
                                out=ctx_acc[gdt][:, n0:n0 + NH], in0=ps_c,
                                scalar=recips[dt],
                                in1=ctx_acc[gdt][:, n0:n0 + NH],
                                op0=ALU.mult, op1=ALU.add)
                if l == 1:
                    # fire this branch's ReduceScatter immediately
                    for dt, (d0, dp) in enumerate(DT[i]):
                        gdt = GBASE[i] + dt
                        nc.sync.dma_start(out=cc_in[i][d0:d0 + dp, :],
                                          in_=ctx_acc[gdt][:, 0:NH])
                        nc.sync.dma_start(out=cc_in[i][C + d0:C + d0 + dp, :],
                                          in_=ctx_acc[gdt][:, NH:NTOK])
                    nc.gpsimd.collective_compute(
                        "ReduceScatter", ALU.add, ins=[cc_in[i].opt()],
                        outs=[cc_out[i].opt()], replica_groups=RG)
                    for dt, (d0, dp) in enumerate(DT[i]):
                        gdt = GBASE[i] + dt
                        t = cpool.tile([dp, NH], f32r, tag=f"ctxr{gdt}",
                                       name=f"ctxr{gdt}")
                        nc.sync.dma_start(
                            out=t, in_=cc_out[i][d0:d0 + dp, :].bitcast(f32r))
                        ctxr[gdt] = t

            for l in range(2):
                if l == 0:
                    wk_sb = wk0_sb
                else:
                    wk_sb = []
                    for kt, (k0, kp) in enumerate(KT):
                        t = kvw.tile([kp, KV], f32r, tag="kvw", name=f"wk1_{kt}")
                        nc.sync.dma_start(out=t,
                                          in_=wkv[l, 0, k0:k0 + kp, :].bitcast(f32r))
                        wk_sb.append(t)
                kb_bc = bvp.tile([128, KV], f32, tag="kbbc", name="kbbc")
                nc.sync.dma_start(out=kb_bc,
                                  in_=_bcast_ap(kvb[l, 0], 0, [[0, 128], [1, KV]]))
                K_sb = [kpool.tile([TP, KV], bf16, tag=f"k{m}", name=f"k{m}")
                        for m in range(NT)]
                for m in range(NT):
                    for (j0, jn) in JSP:
                        ps = ps_proj.tile([TP, 512], f32, tag="proj", name="proj")
                        for kt, (k0, kp) in enumerate(KT):
                            nc.tensor.matmul(
                                ps[:, :jn], eaT[kt][:, m * TP:(m + 1) * TP],
                                wk_sb[kt][:, j0:j0 + jn],
                                start=(kt == 0), stop=(kt == 7))
                        nc.vector.tensor_add(K_sb[m][:, j0:j0 + jn], ps[:, :jn],
                                             kb_bc[:TP, j0:j0 + jn])
                wv_sb = []
                for kt, (k0, kp) in enumerate(KT):
                    t = kvw.tile([kp, KV], f32r, tag="kvw", name=f"wv{l}_{kt}")
                    nc.sync.dma_start(out=t,
                                      in_=wkv[l, 1, k0:k0 + kp, :].bitcast(f32r))
                    wv_sb.append(t)
                VT_sb = [vpool.tile([jp, NTOK], bf16, tag=f"v{jt}", name=f"v{jt}")
                         for jt, (j0, jp) in enumerate(KT)]
                for jt, (j0, jp) in enumerate(KT):
                    vb_col = tiny.tile([jp, 1], f32, tag="vbcol", name="vbcol")
                    nc.sync.dma_start(
                        out=vb_col, in_=_bcast_ap(kvb[l, 1], j0, [[1, jp], [0, 1]]))
                    for nh2 in range(2):
                        n0 = nh2 * NH
                        ps = ps_ctx.tile([jp, NH], f32, tag="ctx", name="ctx")
                        for kt, (k0, kp) in enumerate(KT):
                            nc.tensor.matmul(
                                ps, wv_sb[kt][:, j0:j0 + jp],
                                eaT[kt][:, n0:n0 + NH],
                                start=(kt == 0), stop=(kt == 7))
                        nc.vector.tensor_scalar(
                            out=VT_sb[jt][:, n0:n0 + NH], in0=ps, scalar1=vb_col,
                            scalar2=None, op0=ALU.add)

                pending = None
                for i in BORD:
                    st = emit_q_scores(l, i, K_sb)
                    st["VT_sb"] = VT_sb
                    if pending is not None:
                        emit_tail(pending)
                    pending = st
                emit_tail(pending)

        # ---------------- Phase C: Wo + residual + FFN (token-half) --------
        with contextlib.ExitStack() as phC:
            fw1 = phC.enter_context(tc.tile_pool(name="fw1", bufs=1))
            fw2 = phC.enter_context(tc.tile_pool(name="fw2", bufs=1))
            wow = phC.enter_context(tc.tile_pool(name="wow", bufs=1))
            xpool = phC.enter_context(tc.tile_pool(name="xpool", bufs=2))
            hpool = phC.enter_context(tc.tile_pool(name="hpool", bufs=2))
            htp = phC.enter_context(tc.tile_pool(name="htp", bufs=2))
            h2tp = phC.enter_context(tc.tile_pool(name="h2tp", bufs=1))
            ytp = phC.enter_context(tc.tile_pool(name="ytp", bufs=3))
            opool = phC.enter_context(tc.tile_pool(name="opool", bufs=3))
            epool = phC.enter_context(tc.tile_pool(name="epool", bufs=3))
            tinyc = phC.enter_context(tc.tile_pool(name="tinyc", bufs=2))

            pc_o = phC.enter_context(tc.tile_pool(name="pc_o", bufs=2, space="PSUM"))
            pc_h2 = phC.enter_context(tc.tile_pool(name="pc_h2", bufs=2, space="PSUM"))
            pc_y = phC.enter_context(tc.tile_pool(name="pc_y", bufs=2, space="PSUM"))
            pc_tr = phC.enter_context(tc.tile_pool(name="pc_tr", bufs=2, space="PSUM"))

            # -- stage 1: all weight/bias DMAs (big branches first) --
            W = {}
            for i in BORD:
                C = CS[i]
                JT = [(t * 128, min(128, 4 * C - t * 128))
                      for t in range(4 * C // 128)]
                wo_sb = []
                for dt, (d0, dp) in enumerate(DT[i]):
                    t = wow.tile([dp, C], f32r, tag=f"wo{i}_{dt}",
                                 name=f"wo{i}_{dt}")
                    nc.sync.dma_start(out=t, in_=wo[i][d0:d0 + dp, :].bitcast(f32r))
                    wo_sb.append(t)
                w1_sb = []
                for ct, (c0, cp) in enumerate(DT[i]):
                    t = fw1.tile([cp, 4 * C], bf16, tag=f"w1_{i}_{ct}",
                                 name=f"w1_{i}_{ct}")
                    nc.sync.dma_start(out=t, in_=w1[i][c0:c0 + cp, :])
                    w1_sb.append(t)
                w2_sb = []
                for jt, (j0, jp) in enumerate(JT):
                    t = fw2.tile([jp, C], bf16, tag=f"w2_{i}_{jt}",
                                 name=f"w2_{i}_{jt}")
                    nc.sync.dma_start(out=t, in_=w2[i][j0:j0 + jp, :])
                    w2_sb.append(t)
                b1_sb = []
                for jt, (j0, jp) in enumerate(JT):
                    t = tinyc.tile([jp, 1], f32, tag=f"b1c_{i}_{jt}",
                                   name=f"b1c_{i}_{jt}")
                    nc.sync.dma_start(out=t,
                                      in_=_bcast_ap(bias1[i][j0:j0 + jp], 0,
                                                    [[1, jp], [0, 1]]))
                    b1_sb.append(t)
                b2_sb = []
                for ct, (c0, cp) in enumerate(DT[i]):
                    t = tinyc.tile([cp, 1], f32, tag=f"b2c_{i}_{ct}",
                                   name=f"b2c_{i}_{ct}")
                    nc.sync.dma_start(out=t,
                                      in_=_bcast_ap(bias2[i][c0:c0 + cp], 0,
                                                    [[1, cp], [0, 1]]))
                    b2_sb.append(t)
                W[i] = (wo_sb, w1_sb, w2_sb, b1_sb, b2_sb, JT)

            # -- stage 2: Wo + residual + LN + hT for every branch --
            XH = {}
            for i in BORD:
                C = CS[i]
                ndt = len(DT[i])
                gbase = GBASE[i]
                wo_sb = W[i][0]
                x_sb = []
                hT_sb = [htp.tile([cp, NH], bf16, tag=f"ht{i}_{ct}",
                                  name=f"ht{i}_{ct}")
                         for ct, (c0, cp) in enumerate(DT[i])]
                for m in range(MT):
                    m0 = m * MP
                    ps_o = pc_o.tile([MP, C], f32, tag="o", name="o")
                    for dt, (d0, dp) in enumerate(DT[i]):
                        nc.tensor.matmul(ps_o, ctxr[gbase + dt][:, m0:m0 + MP],
                                         wo_sb[dt], start=(dt == 0),
                                         stop=(dt == ndt - 1))
                    e_t = epool.tile([MP, C], f32, tag="e", name="e")
                    nc.sync.dma_start(out=e_t,
                                      in_=emb_half[m0:m0 + MP, COFF[i]:COFF[i] + C])
                    x_t = xpool.tile([MP, C], f32, tag=f"x{i}_{m}",
                                     name=f"x{i}_{m}", bufs=1)
                    nc.vector.tensor_add(x_t, e_t, ps_o)
                    x_sb.append(x_t)
                    bst = tinyc.tile([MP, 6], f32, tag="bstc", name="bstc")
                    nc.vector.bn_stats(out=bst, in_=x_t)
                    mv = tinyc.tile([MP, 2], f32, tag="mvc", name="mvc")
                    nc.vector.bn_aggr(out=mv, in_=bst)
                    stdv = tinyc.tile([MP, 1], f32, tag="stdc", name="stdc")
                    nc.scalar.activation(stdv, mv[:, 1:2], AF.Sqrt,
                                         bias=eps_ln[:MP])
                    rstd = tinyc.tile([MP, 1], f32, tag="rstdc", name="rstdc")
                    nc.vector.reciprocal(rstd, stdv)
                    h_t = hpool.tile([MP, C], f32, tag="h", name="h")
                    nc.vector.tensor_scalar(out=h_t, in0=x_t, scalar1=mv[:, 0:1],
                                            scalar2=rstd, op0=ALU.subtract,
                                            op1=ALU.mult)
                    for ct, (c0, cp) in enumerate(DT[i]):
                        ps_t3 = pc_tr.tile([cp, MP], f32, tag="tr", name="tr")
                        nc.tensor.transpose(ps_t3, h_t[:, c0:c0 + cp],
                                            ident[:MP, :MP])
                        nc.vector.tensor_copy(hT_sb[ct][:, m0:m0 + MP], ps_t3)
                XH[i] = (x_sb, hT_sb)

            # -- stage 3: fc1+gelu, fc2+bias, transpose back, residual, out --
            for i in BORD:
                C = CS[i]
                ndt = len(DT[i])
                wo_sb, w1_sb, w2_sb, b1_sb, b2_sb, JT = W[i]
                x_sb, hT_sb = XH[i]
                h2T_sb = []
                for jt, (j0, jp) in enumerate(JT):
                    ps_h = pc_h2.tile([jp, NH], f32, tag="h2", name="h2")
                    for ct, (c0, cp) in enumerate(DT[i]):
                        nc.tensor.matmul(ps_h, w1_sb[ct][:, j0:j0 + jp], hT_sb[ct],
                                         start=(ct == 0), stop=(ct == ndt - 1))
                    h2t = h2tp.tile([jp, NH], bf16, tag=f"h2t{jt}",
                                    name=f"h2t{jt}")
                    nc.scalar.activation(h2t, ps_h, AF.Gelu, bias=b1_sb[jt])
                    h2T_sb.append(h2t)
                for ct, (c0, cp) in enumerate(DT[i]):
                    ps_y = pc_y.tile([cp, NH], f32, tag="y", name="y")
                    for jt, (j0, jp) in enumerate(JT):
                        nc.tensor.matmul(ps_y, w2_sb[jt][:, c0:c0 + cp],
                                         h2T_sb[jt],
                                         start=(jt == 0), stop=(jt == len(JT) - 1))
                    yt = ytp.tile([cp, NH], f32, tag="yt", name="yt")
                    nc.scalar.activation(yt, ps_y, AF.Identity, bias=b2_sb[ct])
                    for m in range(MT):
                        m0 = m * MP
                        ps_t4 = pc_tr.tile([MP, cp], f32, tag="tr", name="tr")
                        nc.tensor.transpose(ps_t4, yt[:, m0:m0 + MP],
                                            ident[:cp, :cp])
                        o_t = opool.tile([MP, 128], f32, tag="ot", name="ot")
                        nc.vector.tensor_add(o_t[:, :cp], x_sb[m][:, c0:c0 + cp],
                                             ps_t4)
                        nc.sync.dma_start(out=outs[i][m0:m0 + MP, c0:c0 + cp],
                                          in_=o_t[:, :cp])

    nc.compile()
    return nc


_CACHE = {}


def _get_graph():
    if "nc" not in _CACHE:
        _CACHE["nc"] = build_graph()
    return _CACHE["nc"]


def _prep_core_inputs(inputs, b, g):
    f = np.float32
    emb_cat = np.concatenate(
        [np.asarray(inputs[f"emb{i+1}"][b], dtype=f) for i in range(4)], axis=-1)
    emb_cat = np.ascontiguousarray(emb_cat)
    m = {
        "emb": emb_cat,
        "embT": np.ascontiguousarray(emb_cat.T),
        "emb_half": np.ascontiguousarray(emb_cat[g * NH:(g + 1) * NH]),
    }
    anA_g = np.asarray(inputs["anA_g"], f)
    anA_b = np.asarray(inputs["anA_b"], f)
    wkv_m = np.empty((2, 2, KV, KV), f)
    kvb_m = np.empty((2, 2, KV), f)
    for li in range(2):
        h = 2 * g + li
        Wk = np.asarray(inputs["Wk"][h], f)
        Wv = np.asarray(inputs["Wv"][h], f)
        wkv_m[li, 0] = anA_g[:, None] * Wk.T
        wkv_m[li, 1] = anA_g[:, None] * Wv.T
        kvb_m[li, 0] = anA_b @ Wk.T
        kvb_m[li, 1] = anA_b @ Wv.T
    m["wkv"] = wkv_m
    m["kvb"] = kvb_m
    for i, C in enumerate(CS):
        an_g = np.asarray(inputs[f"an{i+1}_g"], f)
        an_b = np.asarray(inputs[f"an{i+1}_b"], f)
        fn_g = np.asarray(inputs[f"fn{i+1}_g"], f)
        fn_b = np.asarray(inputs[f"fn{i+1}_b"], f)
        Wq = np.asarray(inputs[f"Wq{i+1}"], f)
        wq_i = np.empty((2, C, C), np.float32)
        qb_i = np.empty((2, C), f)
        for li in range(2):
            h = 2 * g + li
            wq_i[li] = an_g[:, None] * Wq[h].T
            qb_i[li] = an_b @ Wq[h].T
        m[f"wq{i}"] = wq_i.astype(ml_dtypes.bfloat16)
        m[f"qb{i}"] = qb_i
        m[f"wo{i}"] = np.ascontiguousarray(np.asarray(inputs[f"Wo{i+1}"], f).T)
        w1_ = np.asarray(inputs[f"fc{i+1}1_w"], f)
        m[f"w1{i}"] = np.ascontiguousarray(fn_g[:, None] * w1_.T).astype(ml_dtypes.bfloat16)
        m[f"b1{i}"] = np.asarray(inputs[f"fc{i+1}1_b"], f) + w1_ @ fn_b
        m[f"w2{i}"] = np.ascontiguousarray(np.asarray(inputs[f"fc{i+1}2_w"], f).T).astype(ml_dtypes.bfloat16)
        m[f"b2{i}"] = np.asarray(inputs[f"fc{i+1}2_b"], f)
    return m


def _run(inputs, trace=False):
    nc = _get_graph()
    in_maps = [_prep_core_inputs(inputs, c // 2, c % 2) for c in range(8)]
    res = run_bass_kernel_spmd(nc, in_maps, list(range(8)), trace=trace)
    full = []
    for i, C in enumerate(CS):
        o = np.empty((B, NTOK, C), np.float32)
        for c in range(8):
            b, g = c // 2, c % 2
            o[b, g * NH:(g + 1) * NH, :] = res.results[c][f"out{i}"]
        full.append(o)
    return tuple(full), res


def kernel(**inputs):
    out, _ = _run(inputs, trace=False)
    return out


def kernel_timed(**inputs):
    out, res = _run(inputs, trace=True)
    return out, res.exec_time_ns


# revision 10
# speedup vs baseline: 1.1686x; 1.0874x over previous
# Trainium2 Bass kernel for nn_Block_ViT (4-branch channel-attention ViT block).
#
# Sharding over 8 cores: core c = 2*b + g handles batch b (of 4) and heads
# {2g, 2g+1} (of 4).  Each core computes K/V/Q projections, channel-attention
# scores, instance-norm + softmax, and its 2-head partial context for all 4
# branches.  Per-branch 2-core ReduceScatters sum the context over heads and
# hand each core one 392-token half; Wo + residual + FFN run token-parallel.
# Host-side prep only reshapes/transposes weights and folds LN affine params
# into adjacent matmuls (algebraically exact).
import sys

sys.path.insert(0, "/opt/trn_rl_repo")

import numpy as np
import ml_dtypes

import concourse.bass as bass
import concourse.tile as tile
from concourse import bacc, mybir
from concourse.bass_utils import run_bass_kernel_spmd
from concourse.masks import make_identity

f32 = mybir.dt.float32
f32r = mybir.dt.float32r
bf16 = mybir.dt.bfloat16
AF = mybir.ActivationFunctionType
ALU = mybir.AluOpType

H = 4
B = 4
NTOK = 784
CS = [64, 128, 256, 512]
COFF = [0, 64, 192, 448]
KV = 960
EPS_LN = 1e-6
EPS_IN = 1e-5
SCALE = 1.0 / float(np.sqrt(np.float32(KV)))

TP = 112          # token tile (attention phase), 7 tiles
NT = 7
NH = 392          # tokens per core after ReduceScatter
MP = 98           # token tile (FFN phase), 4 tiles
MT = 4
KT = [(k * 128, min(128, KV - k * 128)) for k in range(8)]   # 960 = 7*128 + 64
RG = [[0, 1], [2, 3], [4, 5], [6, 7]]
JSP = [(0, 512), (512, 448)]  # bank-aligned split of 960
BORD = [3, 2, 1, 0]           # branch order: biggest first

DT = []
for C in CS:
    DT.append([(t * 128, min(128, C - t * 128)) for t in range((C + 127) // 128)])
GDT = []
for i, C in enumerate(CS):
    for (d0, dp) in DT[i]:
        GDT.append((i, d0, dp, COFF[i] + d0))
GBASE = [sum(len(DT[ii]) for ii in range(i)) for i in range(4)]


def _segments(c0, C):
    segs = []
    r = 0
    while r < C:
        g = c0 + r
        kt = g // 128
        k0 = g - kt * 128
        n = min(KT[kt][1] - k0, C - r, 128 - (r % 128))
        segs.append((kt, k0, r, n))
        r += n
    return segs


def _bcast_ap(src_ap, extra_offset, ap):
    return bass.AP(tensor=src_ap.tensor, offset=src_ap.offset + extra_offset, ap=ap)


def build_graph():
    nc = bacc.Bacc(None, target_bir_lowering=False)

    emb = nc.declare_dram_parameter("emb", [NTOK, KV], f32, isOutput=False)
    embT = nc.declare_dram_parameter("embT", [KV, NTOK], bf16, isOutput=False)
    emb_half = nc.declare_dram_parameter("emb_half", [NH, KV], f32, isOutput=False)
    wkv = nc.declare_dram_parameter("wkv", [2, 2, KV, KV], bf16, isOutput=False)
    kvb = nc.declare_dram_parameter("kvb", [2, 2, KV], f32, isOutput=False)
    wq = [nc.declare_dram_parameter(f"wq{i}", [2, CS[i], CS[i]], bf16, isOutput=False)
          for i in range(4)]
    qb = [nc.declare_dram_parameter(f"qb{i}", [2, CS[i]], f32, isOutput=False)
          for i in range(4)]
    wo = [nc.declare_dram_parameter(f"wo{i}", [CS[i], CS[i]], f32, isOutput=False)
          for i in range(4)]
    w1 = [nc.declare_dram_parameter(f"w1{i}", [CS[i], 4 * CS[i]], bf16, isOutput=False)
          for i in range(4)]
    bias1 = [nc.declare_dram_parameter(f"b1{i}", [4 * CS[i]], f32, isOutput=False)
             for i in range(4)]
    w2 = [nc.declare_dram_parameter(f"w2{i}", [4 * CS[i], CS[i]], bf16, isOutput=False)
          for i in range(4)]
    bias2 = [nc.declare_dram_parameter(f"b2{i}", [CS[i]], f32, isOutput=False)
             for i in range(4)]
    outs = [nc.declare_dram_parameter(f"out{i}", [NH, CS[i]], f32, isOutput=True)
            for i in range(4)]

    import contextlib
    with tile.TileContext(nc) as tc, contextlib.ExitStack() as outer:
        const = outer.enter_context(tc.tile_pool(name="const", bufs=1))
        dram = outer.enter_context(tc.tile_pool(name="dram", bufs=1, space="DRAM"))
        cpool = outer.enter_context(tc.tile_pool(name="cpool", bufs=1))

        ident = const.tile([128, 128], f32, tag="ident", name="ident")
        make_identity(nc, ident)
        ones_col = const.tile([128, 1], f32, tag="ones", name="ones")
        nc.vector.memset(ones_col, 1.0)
        scale_row = const.tile([1, 128], f32, tag="srow", name="srow")
        nc.vector.memset(scale_row, SCALE)
        eps_ln = const.tile([128, 1], f32, tag="epsln", name="epsln")
        nc.vector.memset(eps_ln, EPS_LN)
        eps_in = const.tile([128, 1], f32, tag="epsin", name="epsin")
        nc.vector.memset(eps_in, EPS_IN)

        cc_in = [dram.tile([2 * CS[i], NH], f32, tag=f"ccin{i}", name=f"ccin{i}")
                 for i in range(4)]
        cc_out = [dram.tile([CS[i], NH], f32, tag=f"ccout{i}", name=f"ccout{i}")
                  for i in range(4)]
        stat_dram = dram.tile([10, NTOK], f32, tag="statd", name="statd")
        ctxr = [None] * len(GDT)

        with contextlib.ExitStack() as phA:
            # ------------- persistent attention-phase tiles ----------------
            pA = phA.enter_context(tc.tile_pool(name="pA", bufs=1))
            eaT = [pA.tile([kp, NTOK], bf16, tag=f"eaT{kt}", name=f"eaT{kt}")
                   for kt, (k0, kp) in enumerate(KT)]
            cxT = {}
            for i, C in enumerate(CS):
                cxT[i] = [pA.tile([dp, NTOK], bf16, tag=f"cxT{i}_{t}",
                                  name=f"cxT{i}_{t}")
                          for t, (d0, dp) in enumerate(DT[i])]
            ctx_acc = [pA.tile([dp, NTOK], f32, tag=f"ctxa{g}", name=f"ctxa{g}")
                       for g, (_, _, dp, _) in enumerate(GDT)]

            tiny = phA.enter_context(tc.tile_pool(name="tiny", bufs=4))
            kvw = phA.enter_context(tc.tile_pool(name="kvw", bufs=9))

            ps_misc = phA.enter_context(
                tc.tile_pool(name="ps_misc", bufs=2, space="PSUM"))
            ps_proj = phA.enter_context(
                tc.tile_pool(name="ps_proj", bufs=2, space="PSUM"))
            ps_sc = phA.enter_context(
                tc.tile_pool(name="ps_sc", bufs=1, space="PSUM"))
            ps_ctx = phA.enter_context(
                tc.tile_pool(name="ps_ctx", bufs=2, space="PSUM"))

            # hoisted: head-0 K weights stream in during the LN phase
            wk0_sb = []
            for kt, (k0, kp) in enumerate(KT):
                t = kvw.tile([kp, KV], bf16, tag="kvw", name=f"wk0_{kt}")
                nc.sync.dma_start(out=t, in_=wkv[0, 0, k0:k0 + kp, :])
                wk0_sb.append(t)

            # ------------- Phase A: LayerNorms in transposed domain --------
            ln_stack = contextlib.ExitStack()
            pLN = ln_stack.enter_context(tc.tile_pool(name="pLN", bufs=1))
            emb_pool = ln_stack.enter_context(tc.tile_pool(name="embp", bufs=3))
            ln_tmp = ln_stack.enter_context(tc.tile_pool(name="lntmp", bufs=3))
            bcastp = ln_stack.enter_context(tc.tile_pool(name="bcastp", bufs=3))
            embT_sb = []
            for kt, (k0, kp) in enumerate(KT):
                t = pLN.tile([kp, NTOK], bf16, tag=f"embT{kt}", name=f"embT{kt}")
                nc.sync.dma_start(out=t, in_=embT[k0:k0 + kp, :])
                embT_sb.append(t)
            stat_rows = pLN.tile([10, NTOK], f32, tag="strow", name="strow")

            # stats per n-tile; pack interleaved [m,rstd] pairs per group
            for it in range(NT):
                n0 = it * TP
                et = emb_pool.tile([TP, KV], f32, tag="emb", name="emb")
                nc.sync.dma_start(out=et, in_=emb[n0:n0 + TP, :])
                pack = tiny.tile([TP, 5, 2], f32, tag="pack", name="pack")
                bste = tiny.tile([TP, 2, 6], f32, tag="bste", name="bste")
                nc.vector.bn_stats(out=bste[:, 0, :], in_=et[:, 0:480])
                nc.vector.bn_stats(out=bste[:, 1, :], in_=et[:, 480:960])
                nc.vector.bn_aggr(out=pack[:, 0, :], in_=bste)
                for i, C in enumerate(CS):
                    c0 = COFF[i]
                    bst = tiny.tile([TP, 6], f32, tag="bst", name="bst")
                    nc.vector.bn_stats(out=bst, in_=et[:, c0:c0 + C])
                    nc.vector.bn_aggr(out=pack[:, 1 + i, :], in_=bst)
                # rstd in place on the var column of each pair
                nc.scalar.activation(pack[:, :, 1], pack[:, :, 1], AF.Sqrt,
                                     bias=eps_ln[:TP])
                nc.vector.reciprocal(pack[:, :, 1], pack[:, :, 1])
                ps_t = ps_misc.tile([10, TP], f32, tag="mips", name="mips")
                nc.tensor.transpose(ps_t, pack.rearrange("p a b -> p (a b)"),
                                    ident[:TP, :TP])
                nc.vector.tensor_copy(stat_rows[:, n0:n0 + TP], ps_t)
                nc.sync.dma_start(out=stat_dram[:, n0:n0 + TP],
                                  in_=stat_rows[:, n0:n0 + TP])

            # batched applies per n-chunk (448 + 336 tokens)
            for (h0, hn) in ((0, 448), (448, 336)):
                bc = bcastp.tile([128, 10, hn], f32, tag="bc", name="bc",
                                 bufs=2)
                nc.sync.dma_start(
                    out=bc,
                    in_=_bcast_ap(stat_dram, h0, [[0, 128], [NTOK, 10], [1, hn]]))
                for kt, (k0, kp) in enumerate(KT):
                    tmp = ln_tmp.tile([128, hn], bf16, tag="apt", name="apt")
                    nc.vector.tensor_sub(tmp[:kp, :], embT_sb[kt][:, h0:h0 + hn],
                                         bc[:kp, 0, :])
                    nc.vector.tensor_mul(eaT[kt][:, h0:h0 + hn], tmp[:kp, :],
                                         bc[:kp, 1, :])
                for i, C in enumerate(CS):
                    for (kt, kk0, br, nr) in _segments(COFF[i], C):
                        bt = br // 128
                        r0 = br - bt * 128
                        tmp = ln_tmp.tile([128, hn], bf16, tag="apt", name="apt")
                        nc.vector.tensor_sub(
                            tmp[:nr, :], embT_sb[kt][kk0:kk0 + nr, h0:h0 + hn],
                            bc[kk0:kk0 + nr, 2 + 2 * i, :])
                        nc.vector.tensor_mul(
                            cxT[i][bt][r0:r0 + nr, h0:h0 + hn], tmp[:nr, :],
                            bc[:nr, 3 + 2 * i, :])

            # ------------- Phase A2: attention, branch-pipelined -----------
            ln_stack.close()
            kpool = phA.enter_context(tc.tile_pool(name="kpool", bufs=1))
            vpool = phA.enter_context(tc.tile_pool(name="vpool", bufs=1))
            qpool = phA.enter_context(tc.tile_pool(name="qpool", bufs=1))
            wqp = phA.enter_context(tc.tile_pool(name="wqp", bufs=1))
            scp = phA.enter_context(tc.tile_pool(name="scp", bufs=1))
            expp = phA.enter_context(tc.tile_pool(name="expp", bufs=4))
            ptp = phA.enter_context(tc.tile_pool(name="ptp", bufs=2))
            p0p = phA.enter_context(tc.tile_pool(name="p0p", bufs=4))
            bvp = phA.enter_context(tc.tile_pool(name="bvp", bufs=1))

            def emit_q_scores(l, i, K_sb):
                C = CS[i]
                ndt = len(DT[i])
                wq_sb = []
                for ct, (c0, cp) in enumerate(DT[i]):
                    t = wqp.tile([cp, C], bf16, tag=f"wq{ct}", name=f"wq{ct}")
                    nc.sync.dma_start(out=t, in_=wq[i][l, c0:c0 + cp, :])
                    wq_sb.append(t)
                qb_bc = bvp.tile([128, C], f32, tag="qbbc", name="qbbc")
                nc.sync.dma_start(out=qb_bc,
                                  in_=_bcast_ap(qb[i][l], 0, [[0, 128], [1, C]]))
                Q_sb = [qpool.tile([TP, C], bf16, tag=f"q{m}", name=f"q{m}")
                        for m in range(NT)]
                for m in range(NT):
                    ps = ps_proj.tile([TP, 512], f32, tag="proj", name="proj")
                    for ct, (c0, cp) in enumerate(DT[i]):
                        nc.tensor.matmul(
                            ps[:, :C], cxT[i][ct][:, m * TP:(m + 1) * TP],
                            wq_sb[ct], start=(ct == 0), stop=(ct == ndt - 1))
                    nc.vector.tensor_add(Q_sb[m], ps[:, :C], qb_bc[:TP, :])
                sc_sb = []
                stack = tiny.tile([128, 2 * ndt], f32, tag="stack", name="stack")
                for dt, (d0, dp) in enumerate(DT[i]):
                    ps_s = ps_sc.tile([dp, 1024], f32, tag="sc", name="sc")
                    for (j0, jn) in JSP:
                        for m in range(NT):
                            nc.tensor.matmul(
                                ps_s[:, j0:j0 + jn], Q_sb[m][:, d0:d0 + dp],
                                K_sb[m][:, j0:j0 + jn],
                                start=(m == 0), stop=(m == NT - 1))
                    bsts = tiny.tile([dp, 2, 6], f32, tag="bsts", name="bsts")
                    nc.vector.bn_stats(out=bsts[:, 0, :], in_=ps_s[:, 0:480])
                    nc.vector.bn_stats(out=bsts[:, 1, :], in_=ps_s[:, 480:960])
                    mvs = tiny.tile([dp, 2], f32, tag="mvs", name="mvs")
                    nc.vector.bn_aggr(out=mvs, in_=bsts)
                    nc.vector.tensor_copy(stack[:dp, dt:dt + 1], mvs[:, 0:1])
                    # E[x^2] = mean^2 + var
                    nc.vector.scalar_tensor_tensor(
                        out=stack[:dp, ndt + dt:ndt + dt + 1], in0=mvs[:, 0:1],
                        scalar=mvs[:, 0:1], in1=mvs[:, 1:2],
                        op0=ALU.mult, op1=ALU.add)
                    sc_t = scp.tile([dp, 1024], f32, tag=f"sct{dt}",
                                    name=f"sct{dt}")
                    nc.scalar.activation(sc_t[:, 0:960], ps_s[:, 0:960],
                                         AF.Identity)
                    sc_sb.append(sc_t)
                return dict(l=l, i=i, sc_sb=sc_sb, stack=stack)

            def emit_tail(st):
                l, i, sc_sb, stack = st["l"], st["i"], st["sc_sb"], st["stack"]
                VT_sb = st["VT_sb"]
                C = CS[i]
                ndt = len(DT[i])
                dpc = DT[i][0][1]
                ps_st = ps_misc.tile([1, 2 * ndt], f32, tag="mips", name="mips")
                nc.tensor.matmul(ps_st, ones_col[:dpc, :], stack[:dpc, :],
                                 start=True, stop=True)
                p0 = p0p.tile([1, 2, ndt], f32, tag="p0", name="p0")
                nc.vector.tensor_copy(p0.rearrange("p a b -> p (a b)"), ps_st)
                s1 = p0p.tile([1, 8], f32, tag="p0b", name="p0b")
                # cols: 0=m_ns,1=E2n,2=msq,3=var,4=std,5=a
                nc.vector.tensor_reduce(out=s1[:, 0:2], in_=p0,
                                        axis=mybir.AxisListType.X, op=ALU.add)
                nc.vector.tensor_scalar(out=s1[:, 0:2], in0=s1[:, 0:2],
                                        scalar1=1.0 / C, scalar2=None,
                                        op0=ALU.mult)
                nc.vector.tensor_tensor(out=s1[:, 2:3], in0=s1[:, 0:1],
                                        in1=s1[:, 0:1], op=ALU.mult)
                nc.vector.tensor_scalar(out=s1[:, 3:4], in0=s1[:, 1:2],
                                        scalar1=s1[:, 2:3], scalar2=SCALE * SCALE,
                                        op0=ALU.subtract, op1=ALU.mult)
                nc.scalar.activation(s1[:, 4:5], s1[:, 3:4], AF.Sqrt,
                                     bias=eps_in[:1])
                pair = p0p.tile([1, 2], f32, tag="pair", name="pair")
                nc.vector.reciprocal(pair[:, 0:1], s1[:, 4:5])
                nc.vector.tensor_scalar(out=pair[:, 1:2], in0=s1[:, 0:1],
                                        scalar1=pair[:, 0:1], scalar2=-1.0,
                                        op0=ALU.mult, op1=ALU.mult)
                ps_ab = ps_misc.tile([128, 2], f32, tag="mips", name="mips")
                nc.tensor.matmul(ps_ab, scale_row, pair, start=True, stop=True)
                ab = tiny.tile([128, 2], f32, tag="ab", name="ab")
                nc.vector.tensor_copy(ab, ps_ab)
                pT = [ptp.tile([jp, C], bf16, tag=f"pt{jt}", name=f"pt{jt}")
                      for jt, (j0, jp) in enumerate(KT)]
                recips = []
                for dt, (d0, dp) in enumerate(DT[i]):
                    ex = expp.tile([dp, 960], f32, tag="exp", name="exp")
                    esum = tiny.tile([dp, 1], f32, tag="esum", name="esum")
                    nc.scalar.activation(ex, sc_sb[dt][:, 0:960], AF.Exp,
                                         bias=ab[:dp, 1:2], scale=ab[:dp, 0:1],
                                         accum_out=esum)
                    recip = tiny.tile([dp, 1], f32, tag="recip", name="recip",
                                      bufs=8)
                    nc.vector.tensor_scalar(out=recip, in0=esum, scalar1=4.0,
                                            scalar2=None, op0=ALU.mult)
                    nc.vector.reciprocal(recip, recip)
                    recips.append(recip)
                    for jt, (j0, jp) in enumerate(KT):
                        ps_t2 = ps_misc.tile([jp, dp], f32, tag="mips",
                                             name="mips")
                        nc.tensor.transpose(ps_t2, ex[:, j0:j0 + jp],
                                            ident[:dp, :dp])
                        nc.scalar.copy(pT[jt][:, d0:d0 + dp], ps_t2)
                for dt, (d0, dp) in enumerate(DT[i]):
                    gdt = GBASE[i] + dt
                    for nh2 in range(2):
                        n0 = nh2 * NH
                        ps_c = ps_ctx.tile([dp, NH], f32, tag="ctx", name="ctx")
                        for jt, (j0, jp) in enumerate(KT):
                            nc.tensor.matmul(
                                ps_c, pT[jt][:, d0:d0 + dp],
                                VT_sb[jt][:, n0:n0 + NH],
                                start=(jt == 0), stop=(jt == 7))
                        if l == 0:
                            nc.vector.tensor_scalar(
                                out=ctx_acc[gdt][:, n0:n0 + NH], in0=ps_c,
                                scalar1=recips[dt], scalar2=None, op0=ALU.mult)
                        else:
                            nc.vector.scalar_tensor_tensor(
                                out=ctx_acc[gdt][:, n0:n0 + NH], in0=ps_c,
                                scalar=recips[dt],
                                in1=ctx_acc[gdt][:, n0:n0 + NH],
                                op0=ALU.mult, op1=ALU.add)
                if l == 1:
                    # fire this branch's ReduceScatter immediately
                    for dt, (d0, dp) in enumerate(DT[i]):
                        gdt = GBASE[i] + dt
                        nc.gpsimd.dma_start(out=cc_in[i][d0:d0 + dp, :],
                                            in_=ctx_acc[gdt][:, 0:NH])
                        nc.gpsimd.dma_start(out=cc_in[i][C + d0:C + d0 + dp, :],
                                            in_=ctx_acc[gdt][:, NH:NTOK])
                    nc.gpsimd.collective_compute(
                        "ReduceScatter", ALU.add, ins=[cc_in[i].opt()],
                        outs=[cc_out[i].opt()], replica_groups=RG)
                    for dt, (d0, dp) in enumerate(DT[i]):
                        gdt = GBASE[i] + dt
                        t = cpool.tile([dp, NH], f32r, tag=f"ctxr{gdt}",
                                       name=f"ctxr{gdt}")
                        nc.gpsimd.dma_start(
                            out=t, in_=cc_out[i][d0:d0 + dp, :].bitcast(f32r))
                        ctxr[gdt] = t

            for l in range(2):
                if l == 0:
                    wk_sb = wk0_sb
                else:
                    wk_sb = []
                    for kt, (k0, kp) in enumerate(KT):
                        t = kvw.tile([kp, KV], bf16, tag="kvw", name=f"wk1_{kt}")
                        nc.sync.dma_start(out=t, in_=wkv[l, 0, k0:k0 + kp, :])
                        wk_sb.append(t)
                kb_bc = bvp.tile([128, KV], f32, tag="kbbc", name="kbbc")
                nc.sync.dma_start(out=kb_bc,
                                  in_=_bcast_ap(kvb[l, 0], 0, [[0, 128], [1, KV]]))
                K_sb = [kpool.tile([TP, KV], bf16, tag=f"k{m}", name=f"k{m}")
                        for m in range(NT)]
                for m in range(NT):
                    for (j0, jn) in JSP:
                        ps = ps_proj.tile([TP, 512], f32, tag="proj", name="proj")
                        for kt, (k0, kp) in enumerate(KT):
                            nc.tensor.matmul(
                                ps[:, :jn], eaT[kt][:, m * TP:(m + 1) * TP],
                                wk_sb[kt][:, j0:j0 + jn],
                                start=(kt == 0), stop=(kt == 7))
                        nc.vector.tensor_add(K_sb[m][:, j0:j0 + jn], ps[:, :jn],
                                             kb_bc[:TP, j0:j0 + jn])
                # first branch's Q+scores before the V projection: keeps PE
                # busy while the V weights stream in
                pre_sts = [emit_q_scores(l, BORD[0], K_sb)]
                rest_branches = BORD[1:]
                wv_sb = []
                for kt, (k0, kp) in enumerate(KT):
                    t = kvw.tile([kp, KV], bf16, tag="kvw", name=f"wv{l}_{kt}")
                    nc.sync.dma_start(out=t, in_=wkv[l, 1, k0:k0 + kp, :])
                    wv_sb.append(t)
                VT_sb = [vpool.tile([jp, NTOK], bf16, tag=f"v{jt}", name=f"v{jt}")
                         for jt, (j0, jp) in enumerate(KT)]
                for jt, (j0, jp) in enumerate(KT):
                    vb_col = tiny.tile([jp, 1], f32, tag="vbcol", name="vbcol")
                    nc.sync.dma_start(
                        out=vb_col, in_=_bcast_ap(kvb[l, 1], j0, [[1, jp], [0, 1]]))
                    for nh2 in range(2):
                        n0 = nh2 * NH
                        ps = ps_ctx.tile([jp, NH], f32, tag="ctx", name="ctx")
                        for kt, (k0, kp) in enumerate(KT):
                            nc.tensor.matmul(
                                ps, wv_sb[kt][:, j0:j0 + jp],
                                eaT[kt][:, n0:n0 + NH],
                                start=(kt == 0), stop=(kt == 7))
                        nc.vector.tensor_scalar(
                            out=VT_sb[jt][:, n0:n0 + NH], in0=ps, scalar1=vb_col,
                            scalar2=None, op0=ALU.add)

                queue = list(pre_sts)
                for st in queue:
                    st["VT_sb"] = VT_sb
                for i in rest_branches:
                    st = emit_q_scores(l, i, K_sb)
                    st["VT_sb"] = VT_sb
                    queue.append(st)
                    if len(queue) > 2:
                        emit_tail(queue.pop(0))
                while queue:
                    emit_tail(queue.pop(0))

        # ---------------- Phase C: Wo + residual + FFN (token-half) --------
        with contextlib.ExitStack() as phC:
            fw1 = phC.enter_context(tc.tile_pool(name="fw1", bufs=1))
            fw2 = phC.enter_context(tc.tile_pool(name="fw2", bufs=1))
            wow = phC.enter_context(tc.tile_pool(name="wow", bufs=1))
            xpool = phC.enter_context(tc.tile_pool(name="xpool", bufs=2))
            hpool = phC.enter_context(tc.tile_pool(name="hpool", bufs=2))
            htp = phC.enter_context(tc.tile_pool(name="htp", bufs=2))
            h2tp = phC.enter_context(tc.tile_pool(name="h2tp", bufs=1))
            ytp = phC.enter_context(tc.tile_pool(name="ytp", bufs=3))
            opool = phC.enter_context(tc.tile_pool(name="opool", bufs=3))
            epool = phC.enter_context(tc.tile_pool(name="epool", bufs=3))
            tinyc = phC.enter_context(tc.tile_pool(name="tinyc", bufs=2))

            pc_o = phC.enter_context(tc.tile_pool(name="pc_o", bufs=2, space="PSUM"))
            pc_h2 = phC.enter_context(tc.tile_pool(name="pc_h2", bufs=2, space="PSUM"))
            pc_y = phC.enter_context(tc.tile_pool(name="pc_y", bufs=2, space="PSUM"))
            pc_tr = phC.enter_context(tc.tile_pool(name="pc_tr", bufs=2, space="PSUM"))

            # -- stage 1: all weight/bias DMAs (big branches first) --
            W = {}
            for i in BORD:
                C = CS[i]
                JT = [(t * 128, min(128, 4 * C - t * 128))
                      for t in range(4 * C // 128)]
                wo_sb = []
                for dt, (d0, dp) in enumerate(DT[i]):
                    t = wow.tile([dp, C], f32r, tag=f"wo{i}_{dt}",
                                 name=f"wo{i}_{dt}")
                    nc.sync.dma_start(out=t, in_=wo[i][d0:d0 + dp, :].bitcast(f32r))
                    wo_sb.append(t)
                w1_sb = []
                for ct, (c0, cp) in enumerate(DT[i]):
                    t = fw1.tile([cp, 4 * C], bf16, tag=f"w1_{i}_{ct}",
                                 name=f"w1_{i}_{ct}")
                    nc.sync.dma_start(out=t, in_=w1[i][c0:c0 + cp, :])
                    w1_sb.append(t)
                w2_sb = []
                for jt, (j0, jp) in enumerate(JT):
                    t = fw2.tile([jp, C], bf16, tag=f"w2_{i}_{jt}",
                                 name=f"w2_{i}_{jt}")
                    nc.sync.dma_start(out=t, in_=w2[i][j0:j0 + jp, :])
                    w2_sb.append(t)
                b1_sb = []
                for jt, (j0, jp) in enumerate(JT):
                    t = tinyc.tile([jp, 1], f32, tag=f"b1c_{i}_{jt}",
                                   name=f"b1c_{i}_{jt}")
                    nc.sync.dma_start(out=t,
                                      in_=_bcast_ap(bias1[i][j0:j0 + jp], 0,
                                                    [[1, jp], [0, 1]]))
                    b1_sb.append(t)
                b2_sb = []
                for ct, (c0, cp) in enumerate(DT[i]):
                    t = tinyc.tile([cp, 1], f32, tag=f"b2c_{i}_{ct}",
                                   name=f"b2c_{i}_{ct}")
                    nc.sync.dma_start(out=t,
                                      in_=_bcast_ap(bias2[i][c0:c0 + cp], 0,
                                                    [[1, cp], [0, 1]]))
                    b2_sb.append(t)
                W[i] = (wo_sb, w1_sb, w2_sb, b1_sb, b2_sb, JT)

            # -- stage 2: Wo + residual + LN + hT for every branch --
            XH = {}
            for i in BORD:
                C = CS[i]
                ndt = len(DT[i])
                gbase = GBASE[i]
                wo_sb = W[i][0]
                x_sb = []
                hT_sb = [htp.tile([cp, NH], bf16, tag=f"ht{i}_{ct}",
                                  name=f"ht{i}_{ct}")
                         for ct, (c0, cp) in enumerate(DT[i])]
                for m in range(MT):
                    m0 = m * MP
                    ps_o = pc_o.tile([MP, C], f32, tag="o", name="o")
                    for dt, (d0, dp) in enumerate(DT[i]):
                        nc.tensor.matmul(ps_o, ctxr[gbase + dt][:, m0:m0 + MP],
                                         wo_sb[dt], start=(dt == 0),
                                         stop=(dt == ndt - 1))
                    e_t = epool.tile([MP, C], f32, tag="e", name="e")
                    nc.sync.dma_start(out=e_t,
                                      in_=emb_half[m0:m0 + MP, COFF[i]:COFF[i] + C])
                    x_t = xpool.tile([MP, C], f32, tag=f"x{i}_{m}",
                                     name=f"x{i}_{m}", bufs=1)
                    nc.vector.tensor_add(x_t, e_t, ps_o)
                    x_sb.append(x_t)
                    bst = tinyc.tile([MP, 6], f32, tag="bstc", name="bstc")
                    nc.vector.bn_stats(out=bst, in_=x_t)
                    mv = tinyc.tile([MP, 2], f32, tag="mvc", name="mvc")
                    nc.vector.bn_aggr(out=mv, in_=bst)
                    stdv = tinyc.tile([MP, 1], f32, tag="stdc", name="stdc")
                    nc.scalar.activation(stdv, mv[:, 1:2], AF.Sqrt,
                                         bias=eps_ln[:MP])
                    rstd = tinyc.tile([MP, 1], f32, tag="rstdc", name="rstdc")
                    nc.vector.reciprocal(rstd, stdv)
                    h_t = hpool.tile([MP, C], f32, tag="h", name="h")
                    nc.vector.tensor_scalar(out=h_t, in0=x_t, scalar1=mv[:, 0:1],
                                            scalar2=rstd, op0=ALU.subtract,
                                            op1=ALU.mult)
                    for ct, (c0, cp) in enumerate(DT[i]):
                        ps_t3 = pc_tr.tile([cp, MP], f32, tag="tr", name="tr")
                        nc.tensor.transpose(ps_t3, h_t[:, c0:c0 + cp],
                                            ident[:MP, :MP])
                        nc.scalar.copy(hT_sb[ct][:, m0:m0 + MP], ps_t3)
                XH[i] = (x_sb, hT_sb)

            # -- stage 3: fc1+gelu, fc2+bias, transpose back, residual, out --
            for i in BORD:
                C = CS[i]
                ndt = len(DT[i])
                wo_sb, w1_sb, w2_sb, b1_sb, b2_sb, JT = W[i]
                x_sb, hT_sb = XH[i]
                h2T_sb = []
                for jt, (j0, jp) in enumerate(JT):
                    ps_h = pc_h2.tile([jp, NH], f32, tag="h2", name="h2")
                    for ct, (c0, cp) in enumerate(DT[i]):
                        nc.tensor.matmul(ps_h, w1_sb[ct][:, j0:j0 + jp], hT_sb[ct],
                                         start=(ct == 0), stop=(ct == ndt - 1))
                    h2t = h2tp.tile([jp, NH], bf16, tag=f"h2t{jt}",
                                    name=f"h2t{jt}")
                    nc.scalar.activation(h2t, ps_h, AF.Gelu, bias=b1_sb[jt])
                    h2T_sb.append(h2t)
                for ct, (c0, cp) in enumerate(DT[i]):
                    ps_y = pc_y.tile([cp, NH], f32, tag="y", name="y")
                    for jt, (j0, jp) in enumerate(JT):
                        nc.tensor.matmul(ps_y, w2_sb[jt][:, c0:c0 + cp],
                                         h2T_sb[jt],
                                         start=(jt == 0), stop=(jt == len(JT) - 1))
                    yt = ytp.tile([cp, NH], f32, tag="yt", name="yt")
                    nc.scalar.activation(yt, ps_y, AF.Identity, bias=b2_sb[ct])
                    for m in range(MT):
                        m0 = m * MP
                        ps_t4 = pc_tr.tile([MP, cp], f32, tag="tr", name="tr")
                        nc.tensor.transpose(ps_t4, yt[:, m0:m0 + MP],
                                            ident[:cp, :cp])
                        o_t = opool.tile([MP, 128], f32, tag="ot", name="ot")
                        nc.vector.tensor_add(o_t[:, :cp], x_sb[m][:, c0:c0 + cp],
                                             ps_t4)
                        nc.sync.dma_start(out=outs[i][m0:m0 + MP, c0:c0 + cp],
                                          in_=o_t[:, :cp])

    nc.compile()
    return nc


_CACHE = {}


def _get_graph():
    if "nc" not in _CACHE:
        _CACHE["nc"] = build_graph()
    return _CACHE["nc"]


def _prep_core_inputs(inputs, b, g):
    f = np.float32
    emb_cat = np.concatenate(
        [np.asarray(inputs[f"emb{i+1}"][b], dtype=f) for i in range(4)], axis=-1)
    emb_cat = np.ascontiguousarray(emb_cat)
    m = {
        "emb": emb_cat,
        "embT": np.ascontiguousarray(emb_cat.T).astype(ml_dtypes.bfloat16),
        "emb_half": np.ascontiguousarray(emb_cat[g * NH:(g + 1) * NH]),
    }
    anA_g = np.asarray(inputs["anA_g"], f)
    anA_b = np.asarray(inputs["anA_b"], f)
    wkv_m = np.empty((2, 2, KV, KV), f)  # converted to bf16 below
    kvb_m = np.empty((2, 2, KV), f)
    for li in range(2):
        h = 2 * g + li
        Wk = np.asarray(inputs["Wk"][h], f)
        Wv = np.asarray(inputs["Wv"][h], f)
        wkv_m[li, 0] = anA_g[:, None] * Wk.T
        wkv_m[li, 1] = anA_g[:, None] * Wv.T
        kvb_m[li, 0] = anA_b @ Wk.T
        kvb_m[li, 1] = anA_b @ Wv.T
    m["wkv"] = wkv_m.astype(ml_dtypes.bfloat16)
    m["kvb"] = kvb_m
    for i, C in enumerate(CS):
        an_g = np.asarray(inputs[f"an{i+1}_g"], f)
        an_b = np.asarray(inputs[f"an{i+1}_b"], f)
        fn_g = np.asarray(inputs[f"fn{i+1}_g"], f)
        fn_b = np.asarray(inputs[f"fn{i+1}_b"], f)
        Wq = np.asarray(inputs[f"Wq{i+1}"], f)
        wq_i = np.empty((2, C, C), np.float32)
        qb_i = np.empty((2, C), f)
        for li in range(2):
            h = 2 * g + li
            wq_i[li] = an_g[:, None] * Wq[h].T
            qb_i[li] = an_b @ Wq[h].T
        m[f"wq{i}"] = wq_i.astype(ml_dtypes.bfloat16)
        m[f"qb{i}"] = qb_i
        m[f"wo{i}"] = np.ascontiguousarray(np.asarray(inputs[f"Wo{i+1}"], f).T)
        w1_ = np.asarray(inputs[f"fc{i+1}1_w"], f)
        m[f"w1{i}"] = np.ascontiguousarray(fn_g[:, None] * w1_.T).astype(ml_dtypes.bfloat16)
        m[f"b1{i}"] = np.asarray(inputs[f"fc{i+1}1_b"], f) + w1_ @ fn_b
        m[f"w2{i}"] = np.ascontiguousarray(np.asarray(inputs[f"fc{i+1}2_w"], f).T).astype(ml_dtypes.bfloat16)
        m[f"b2{i}"] = np.asarray(inputs[f"fc{i+1}2_b"], f)
    return m


def _run(inputs, trace=False):
    nc = _get_graph()
    in_maps = [_prep_core_inputs(inputs, c // 2, c % 2) for c in range(8)]
    res = run_bass_kernel_spmd(nc, in_maps, list(range(8)), trace=trace)
    full = []
    for i, C in enumerate(CS):
        o = np.empty((B, NTOK, C), np.float32)
        for c in range(8):
            b, g = c // 2, c % 2
            o[b, g * NH:(g + 1) * NH, :] = res.results[c][f"out{i}"]
        full.append(o)
    return tuple(full), res


def kernel(**inputs):
    out, _ = _run(inputs, trace=False)
    return out


def kernel_timed(**inputs):
    out, res = _run(inputs, trace=True)
    return out, res.exec_time_ns
